# revision 17
# baseline (speedup 1.0000x reference)
"""Trainium2 Bass kernel for the neural-CDE discriminator.

Reference computation (B=1024, T=64, C=17, HIDDEN=64):
  h0 = init_mlp(ys[:, 0])                    (17 -> 64 -> 64 -> 64, lipswish/sigmoid)
  63 reversible-Heun steps; each step evaluates the func MLP twice:
      vf(t, h) = tanh(mlp([t, h]))           (65 -> 64 -> 64 -> 1088)
      f(t, h, dx) = einsum('bhc,bc->bh', vf.reshape(B, 64, 17), dx)
  score = y_T @ rW + rb; output = mean(score)

Key algebraic restructuring (exact, same arithmetic):
  - f1 of step t and f0 of step t+1 are the same evaluation (same time t+1,
    same state z1, and dX1[t] == dX0[t+1]), so the reference's 126 MLP evals
    reduce to 64 distinct ones.
  - With u = 2y, the reversible-Heun update collapses to a leapfrog:
        z_{k+1} = z_{k-1} + 2 * f(k, z_k, dX0[k])       (k = 1..62)
        z_1     = z_0 + f(0, z_0, dX0[0])
        u_63    = z_62 + z_63 + f(63, z_63, dX0[62])
    Each state update is absorbed into the einsum accumulation chain's seed
    (dx pre-scaled by 2), so one eval costs exactly 17 fused MAC ops on DVE.
  - 0.5 from the Heun average is folded into the readout weights; lipswish's
    0.909 into the next layer's weights; time t enters through a precomputed
    per-eval L1 bias table; the L3 bias is added by a K=1 ones matmul that
    accumulates into PSUM before the main L3 matmul.

Sharding: pure data parallel, batch 1024 -> 128 per NeuronCore on 8 cores.

Layout per core: L1/L2 activations feature-major [feat, 128] (stationary
weights, no transposes); L3 batch-major via lhsT = activations, rhs = W3 with
columns permuted channel-outer (o' = c*64 + h) and rounded to float32r on
device for the fast fp32 PE path. The per-sample contraction over c runs as
17 scalar_tensor_tensor MACs with per-partition dx scalars. One PE transpose
+ copy per eval turns the new state batch-major -> feature-major.
"""

import sys

for _p in ("/opt/trn_rl_repo",):
    if _p not in sys.path:
        sys.path.append(_p)

import numpy as np

import concourse.bass as bass
import concourse.mybir as mybir
import concourse.tile as tile
from concourse import bacc
from concourse.bass_utils import run_bass_kernel_spmd
from concourse.masks import make_identity

B = 1024
T = 64
C = 17
H = 64
NCORES = 8
BC = B // NCORES          # 128 samples per core
NSTEP = T - 1             # 63
NEVAL = T                 # 64 distinct MLP evaluations
L3 = H * C                # 1088
LIP = 0.909

F32 = mybir.dt.float32
F32R = mybir.dt.float32r
AF = mybir.ActivationFunctionType
ALU = mybir.AluOpType

L3_F32R = True


def _build_program():
    nc = bacc.Bacc("TRN2", target_bir_lowering=False, debug=False)

    # ---- DRAM I/O -------------------------------------------------------
    ys_d = nc.dram_tensor("ys", [BC, T * C], F32, kind="ExternalInput")
    f1hat_d = nc.dram_tensor("f1hat", [H, H], F32, kind="ExternalInput")
    biasa_d = nc.dram_tensor("biasa", [H, NEVAL], F32, kind="ExternalInput")
    f2p_d = nc.dram_tensor("f2p", [H, H], F32, kind="ExternalInput")
    b2f_d = nc.dram_tensor("b2f", [H, 1], F32, kind="ExternalInput")
    f3e_d = nc.dram_tensor("f3e", [H + 1, L3], F32, kind="ExternalInput")
    i1_d = nc.dram_tensor("i1", [C, H], F32, kind="ExternalInput")
    b1i_d = nc.dram_tensor("b1i", [H, 1], F32, kind="ExternalInput")
    i2s_d = nc.dram_tensor("i2s", [H, H], F32, kind="ExternalInput")
    b2i_d = nc.dram_tensor("b2i", [H, 1], F32, kind="ExternalInput")
    i3s_d = nc.dram_tensor("i3s", [H, H], F32, kind="ExternalInput")
    b3ih_d = nc.dram_tensor("b3ih", [H, 1], F32, kind="ExternalInput")
    rwh_d = nc.dram_tensor("rwh", [1, H], F32, kind="ExternalInput")
    out_d = nc.dram_tensor("partial", [1, 1], F32, kind="ExternalOutput")

    # ---- persistent SBUF ------------------------------------------------
    ys_t = nc.alloc_sbuf_tensor("ys_t", [BC, T * C], F32).ap()
    diff_t = nc.alloc_sbuf_tensor("diff_t", [BC, NSTEP * C], F32).ap()
    diff2_t = nc.alloc_sbuf_tensor("diff2_t", [BC, NSTEP * C], F32).ap()
    f1hat_t = nc.alloc_sbuf_tensor("f1hat_t", [H, H], F32).ap()
    biasa_t = nc.alloc_sbuf_tensor("biasa_t", [H, NEVAL], F32).ap()
    f2p_t = nc.alloc_sbuf_tensor("f2p_t", [H, H], F32).ap()
    b2f_t = nc.alloc_sbuf_tensor("b2f_t", [H, 1], F32).ap()
    f3e_t = nc.alloc_sbuf_tensor("f3e_t", [H + 1, L3], F32).ap()
    i1_t = nc.alloc_sbuf_tensor("i1_t", [C, H], F32).ap()
    b1i_t = nc.alloc_sbuf_tensor("b1i_t", [H, 1], F32).ap()
    i2s_t = nc.alloc_sbuf_tensor("i2s_t", [H, H], F32).ap()
    b2i_t = nc.alloc_sbuf_tensor("b2i_t", [H, 1], F32).ap()
    i3s_t = nc.alloc_sbuf_tensor("i3s_t", [H, H], F32).ap()
    b3ih_t = nc.alloc_sbuf_tensor("b3ih_t", [H, 1], F32).ap()
    rwh_t = nc.alloc_sbuf_tensor("rwh_t", [1, H], F32).ap()
    ident_t = nc.alloc_sbuf_tensor("ident_t", [128, 128], F32).ap()
    ones_t = nc.alloc_sbuf_tensor("ones_t", [BC, 1], F32).ap()
    onesr_t = nc.alloc_sbuf_tensor("onesr_t", [1, BC], F32).ap()

    l3dt = F32R if L3_F32R else F32
    if L3_F32R:
        f3e_r = nc.alloc_sbuf_tensor("f3e_r", [H + 1, L3], F32R).ap()
        ones1_r = nc.alloc_sbuf_tensor("ones1_r", [1, BC], F32R).ap()
    else:
        f3e_r = f3e_t
        ones1_r = onesr_t

    with tile.TileContext(nc) as tc:
        with (
            tc.tile_pool(name="psA", bufs=2, space="PSUM") as psA,
            tc.tile_pool(name="psB", bufs=2, space="PSUM") as psB,
            tc.tile_pool(name="sb_a1", bufs=2) as sb_a1,
            tc.tile_pool(name="sb_a2", bufs=2) as sb_a2,
            tc.tile_pool(name="sb_a3", bufs=2) as sb_a3,
            tc.tile_pool(name="sb_acc", bufs=3) as sb_acc,
            tc.tile_pool(name="sb_st", bufs=6) as sb_st,
            tc.tile_pool(name="sb_zf", bufs=2) as sb_zf,
            tc.tile_pool(name="sb_fin", bufs=2) as sb_fin,
        ):
            # ---- loads -------------------------------------------------
            for i in range(4):
                sl = slice(i * (BC // 4), (i + 1) * (BC // 4))
                nc.sync.dma_start(ys_t[sl, :], ys_d[sl, :])
            nc.sync.dma_start(f3e_t[0:33, :], f3e_d[0:33, :])
            nc.sync.dma_start(f3e_t[33:65, :], f3e_d[33:65, :])
            for dst, src in (
                (f1hat_t, f1hat_d), (biasa_t, biasa_d), (f2p_t, f2p_d),
                (b2f_t, b2f_d), (i1_t, i1_d), (b1i_t, b1i_d),
                (i2s_t, i2s_d), (b2i_t, b2i_d), (i3s_t, i3s_d),
                (b3ih_t, b3ih_d), (rwh_t, rwh_d),
            ):
                nc.sync.dma_start(dst[:, :], src[:, :])

            make_identity(nc, ident_t)
            nc.gpsimd.memset(ones_t, 1.0)
            nc.gpsimd.memset(onesr_t, 1.0)
            if L3_F32R:
                # round the L3 weights+bias and the ones row to f32r once
                nc.scalar.copy(f3e_r[:, :], f3e_t[:, :])
                nc.scalar.copy(ones1_r[:, :], onesr_t[:, :])

            # diff[b, t*17+c] = ys[b, (t+1)*17+c] - ys[b, t*17+c]
            nc.vector.tensor_tensor(
                out=diff_t[:, :], in0=ys_t[:, C:], in1=ys_t[:, : NSTEP * C],
                op=ALU.subtract,
            )
            nc.vector.tensor_scalar_mul(diff2_t[:, :], diff_t[:, :], 2.0)

            # ---- init MLP: h0 = sigmoid(mlp3(ys[:, 0])) ----------------
            ps = psA.tile([C, BC], F32, tag="ps_s")
            nc.tensor.transpose(ps[:, :], ys_t[:, 0:C], ident_t[:, :])
            x0 = sb_a1.tile([C, BC], F32, tag="a1")
            nc.scalar.copy(x0[:, :], ps[:, :])

            ps1 = psA.tile([H, BC], F32, tag="ps_s")
            nc.tensor.matmul(ps1[:, :], i1_t[:, :], x0[:, :],
                             start=True, stop=True)
            a1 = sb_a1.tile([H, BC], F32, tag="a1")
            nc.scalar.activation(a1[:, :], ps1[:, :], AF.Silu, bias=b1i_t[:, :])

            ps2 = psA.tile([H, BC], F32, tag="ps_s")
            nc.tensor.matmul(ps2[:, :], i2s_t[:, :], a1[:, :],
                             start=True, stop=True)
            a2 = sb_a1.tile([H, BC], F32, tag="a1")
            nc.scalar.activation(a2[:, :], ps2[:, :], AF.Silu, bias=b2i_t[:, :])

            ps3 = psA.tile([H, BC], F32, tag="ps_s")
            nc.tensor.matmul(ps3[:, :], i3s_t[:, :], a2[:, :],
                             start=True, stop=True)
            # sigmoid(x) = 0.5 + 0.5*tanh(x/2); bias input is 0.5*b3i
            th = sb_a1.tile([H, BC], F32, tag="a1")
            nc.scalar.activation(th[:, :], ps3[:, :], AF.Tanh,
                                 bias=b3ih_t[:, :], scale=0.5)
            h0f = sb_zf.tile([H, BC], F32, tag="zf")
            nc.vector.tensor_scalar(h0f[:, :], th[:, :], 0.5, 0.5,
                                    ALU.mult, ALU.add)

            # h0 batch-major
            psh = psA.tile([BC, H], F32, tag="ps_s")
            nc.tensor.transpose(psh[:, :], h0f[:, :], ident_t[0:H, 0:H])
            h0bm = sb_st.tile([BC, H], F32, tag="st")
            nc.scalar.copy(h0bm[:, :], psh[:, :])

            # ---- one func-MLP evaluation + fused state MAC chain -------
            def eval_step(rhs_feat, e, dsrc, dcol, seed):
                """tanh-MLP at time e on state rhs_feat, then
                out = seed + sum_c vf[:, c, :] * dsrc[:, dcol+c]."""
                ps1 = psA.tile([H, BC], F32, tag="ps_s")
                nc.tensor.matmul(ps1[:, :], f1hat_t[:, :], rhs_feat[:, :],
                                 start=True, stop=True)
                a1 = sb_a1.tile([H, BC], F32, tag="a1")
                nc.scalar.activation(a1[:, :], ps1[:, :], AF.Silu,
                                     bias=biasa_t[:, e:e + 1])
                ps2 = psA.tile([H, BC], F32, tag="ps_s")
                nc.tensor.matmul(ps2[:, :], f2p_t[:, :], a1[:, :],
                                 start=True, stop=True)
                a2e = sb_a2.tile([H + 1, BC], l3dt, tag="a2e")
                nc.scalar.activation(a2e[0:H, :], ps2[:, :], AF.Silu,
                                     bias=b2f_t[:, :])
                # homogeneous ones row folds the L3 bias into the GEMM
                nc.vector.tensor_copy(a2e[H:H + 1, :], ones1_r[:, :])

                ps3 = psB.tile([BC, 1536], F32, tag="ps_b")
                for n0, nw in ((0, 512), (512, 512), (1024, 64)):
                    nc.tensor.matmul(ps3[:, n0:n0 + nw], a2e[:, :],
                                     f3e_r[:, n0:n0 + nw],
                                     start=True, stop=True)
                a3a = sb_a3.tile([BC, 512], F32, tag="a3a")
                a3b = sb_a3.tile([BC, L3 - 512], F32, tag="a3b")
                nc.scalar.activation(a3a[:, :], ps3[:, 0:512], AF.Tanh)
                nc.scalar.activation(a3b[:, :], ps3[:, 512:L3], AF.Tanh)

                acc = seed
                dst = None
                for c in range(C):
                    if c < 8:
                        src = a3a[:, c * H:(c + 1) * H]
                    else:
                        src = a3b[:, (c - 8) * H:(c - 7) * H]
                    if c == C - 1:
                        dst = sb_st.tile([BC, H], F32, tag="st")
                    else:
                        dst = sb_acc.tile([BC, H], F32, tag="acc")
                    nc.vector.scalar_tensor_tensor(
                        out=dst[:, :], in0=src,
                        scalar=dsrc[:, dcol + c:dcol + c + 1],
                        in1=acc[:, :] if not isinstance(acc, bass.AP) else acc,
                        op0=ALU.mult, op1=ALU.add,
                    )
                    acc = dst
                return dst

            def to_feat(z_bm):
                pst = psA.tile([H, BC], F32, tag="ps_s")
                nc.tensor.transpose(pst[:, :], z_bm[:, :], ident_t[:, :])
                z_feat = sb_zf.tile([H, BC], F32, tag="zf")
                nc.scalar.copy(z_feat[:, :], pst[:, :])
                return z_feat

            # ---- leapfrog recurrence -----------------------------------
            # eval 0: z_1 = z_0 + f(0, z_0, dX0[0])
            z_prev_bm = h0bm                       # z_0
            z_cur_bm = eval_step(h0f, 0, diff_t, 0, h0bm)   # z_1
            z_cur_feat = to_feat(z_cur_bm)

            for k in range(1, NSTEP):              # k = 1..62
                # z_{k+1} = z_{k-1} + 2 * f(k, z_k, dX0[k])
                z_next = eval_step(z_cur_feat, k, diff2_t, k * C, z_prev_bm)
                z_prev_bm = z_cur_bm
                z_cur_bm = z_next
                z_cur_feat = to_feat(z_cur_bm)

            # eval 63: u_63 = z_62 + z_63 + f(63, z_63, dX0[62])
            seed_u = sb_acc.tile([BC, H], F32, tag="seed")
            nc.vector.scalar_tensor_tensor(
                out=seed_u[:, :], in0=z_prev_bm[:, :], scalar=1.0,
                in1=z_cur_bm[:, :], op0=ALU.mult, op1=ALU.add,
            )
            u_bm = eval_step(z_cur_feat, NSTEP, diff_t, (NSTEP - 1) * C,
                             seed_u)

            # ---- readout: partial = sum_b sum_h u[b,h] * rW[h]/2 -------
            psr = psA.tile([1, H], F32, tag="ps_s")
            nc.tensor.matmul(psr[:, :], ones_t[:, :], u_bm[:, :],
                             start=True, stop=True)
            r0 = sb_fin.tile([1, H], F32)
            nc.scalar.copy(r0[:, :], psr[:, :])
            r1 = sb_fin.tile([1, H], F32)
            nc.vector.tensor_tensor(out=r1[:, :], in0=r0[:, :], in1=rwh_t[:, :],
                                    op=ALU.mult)
            r2 = sb_fin.tile([1, 1], F32)
            nc.vector.tensor_reduce(out=r2[:, :], in_=r1[:, :],
                                    axis=mybir.AxisListType.X, op=ALU.add)
            nc.sync.dma_start(out_d[:, :], r2[:, :])

    nc.finalize()
    return nc


_CACHE = {}


def _get_program():
    if "nc" not in _CACHE:
        _CACHE["nc"] = _build_program()
    return _CACHE["nc"]


def _prep_consts(init_params, func_params, readout_W):
    (i1w, i1b), (i2w, i2b), (i3w, i3b) = [
        (np.asarray(w, np.float32), np.asarray(b, np.float32))
        for w, b in init_params
    ]
    (f1w, f1b), (f2w, f2b), (f3w, f3b) = [
        (np.asarray(w, np.float32), np.asarray(b, np.float32))
        for w, b in func_params
    ]
    rw = np.asarray(readout_W, np.float32).reshape(H)

    f1hat = np.ascontiguousarray(f1w[1:, :])            # [64, 64]
    w_t = f1w[0, :]                                     # [64]
    t_e = np.arange(NEVAL, dtype=np.float32)            # eval k happens at t=k
    biasa = f1b[:, None] + np.outer(w_t, t_e)           # [64, 64]

    f2p = (LIP * f2w).astype(np.float32)

    # permute columns: f3perm[:, c*64+h] = f3w[:, h*17+c]
    f3perm = f3w.reshape(H, H, C).transpose(0, 2, 1).reshape(H, L3)
    b3perm = f3b.reshape(H, C).T.reshape(L3)
    f3e = np.concatenate([LIP * f3perm, b3perm[None, :]], axis=0)  # [65, 1088]

    i2s = (LIP * i2w).astype(np.float32)
    i3s = (LIP * i3w).astype(np.float32)
    b3ih = (0.5 * i3b).astype(np.float32)

    rwh = (0.5 * rw).astype(np.float32)[None, :]        # [1, 64]

    return {
        "f1hat": np.ascontiguousarray(f1hat, np.float32),
        "biasa": np.ascontiguousarray(biasa, np.float32),
        "f2p": np.ascontiguousarray(f2p, np.float32),
        "b2f": np.ascontiguousarray(f2b.reshape(H, 1), np.float32),
        "f3e": np.ascontiguousarray(f3e, np.float32),
        "i1": np.ascontiguousarray(i1w, np.float32),
        "b1i": np.ascontiguousarray(i1b.reshape(H, 1), np.float32),
        "i2s": np.ascontiguousarray(i2s, np.float32),
        "b2i": np.ascontiguousarray(i2b.reshape(H, 1), np.float32),
        "i3s": np.ascontiguousarray(i3s, np.float32),
        "b3ih": np.ascontiguousarray(b3ih.reshape(H, 1), np.float32),
        "rwh": np.ascontiguousarray(rwh, np.float32),
    }


def kernel(ys_coeffs, init_params, func_params, readout_W, readout_b,
           _trace=False, _tmpdir=None):
    ys = np.asarray(ys_coeffs, np.float32)
    assert ys.shape == (B, T, C), ys.shape
    rb = float(np.asarray(readout_b, np.float32).reshape(-1)[0])

    consts = _prep_consts(init_params, func_params, readout_W)
    nc = _get_program()

    in_maps = []
    for cid in range(NCORES):
        m = dict(consts)
        m["ys"] = np.ascontiguousarray(
            ys[cid * BC:(cid + 1) * BC].reshape(BC, T * C))
        in_maps.append(m)

    kw = {}
    if _trace:
        kw = dict(trace=True, tmpdir=_tmpdir)
    res = run_bass_kernel_spmd(nc, in_maps, core_ids=list(range(NCORES)), **kw)
    total = sum(float(r["partial"][0, 0]) for r in res.results)
    out = np.float32(total / B + rb)
    if _trace:
        return np.asarray(out, np.float32), res
    return np.asarray(out, np.float32)


# revision 20
# speedup vs baseline: 1.2967x; 1.2967x over previous
"""Trainium2 Bass kernel for the neural-CDE discriminator.

Reference computation (B=1024, T=64, C=17, HIDDEN=64):
  h0 = init_mlp(ys[:, 0])                    (17 -> 64 -> 64 -> 64, lipswish/sigmoid)
  63 reversible-Heun steps; each step evaluates the func MLP twice:
      vf(t, h) = tanh(mlp([t, h]))           (65 -> 64 -> 64 -> 1088)
      f(t, h, dx) = einsum('bhc,bc->bh', vf.reshape(B, 64, 17), dx)
  score = y_T @ rW + rb; output = mean(score)

Key algebraic restructuring (exact, same arithmetic):
  - f1 of step t and f0 of step t+1 are the same evaluation (same time t+1,
    same state z1, and dX1[t] == dX0[t+1]), so the reference's 126 MLP evals
    reduce to 64 distinct ones.
  - With u = 2y, the reversible-Heun update collapses to a leapfrog:
        z_{k+1} = z_{k-1} + 2 * f(k, z_k, dX0[k])       (k = 1..62)
        z_1     = z_0 + f(0, z_0, dX0[0])
        u_63    = z_62 + z_63 + f(63, z_63, dX0[62])
    Each state update is absorbed into the einsum's seed (dx pre-scaled
    by 2).
  - 0.5 from the Heun average is folded into the readout weights; lipswish's
    0.909 into the next layer's weights; time t enters through a precomputed
    per-eval L1 bias table; the L3 bias rides a homogeneous ones row.

Sharding: pure data parallel, batch 1024 -> 128 per NeuronCore on 8 cores.

Layout per core: L1/L2 activations feature-major [feat, 128] (stationary
weights, no transposes); L3 batch-major via lhsT = activations, rhs = W3 with
columns permuted channel-outer (o' = c*64 + h). Matmul inputs are rounded to
MM_DT (bf16 by default) on device; accumulation stays fp32 in PSUM. The
per-sample contraction over c runs as two broadcast-AP multiplies (pipelined
against the two tanh chunks) + one strided tensor_reduce + a seed add, all
fp32 on the vector engine. One PE transpose + copy per eval turns the new
state batch-major -> feature-major.
"""

import sys

for _p in ("/opt/trn_rl_repo",):
    if _p not in sys.path:
        sys.path.append(_p)

import numpy as np

import concourse.bass as bass
import concourse.mybir as mybir
import concourse.tile as tile
from concourse import bacc
from concourse.bass_utils import run_bass_kernel_spmd
from concourse.masks import make_identity

B = 1024
T = 64
C = 17
H = 64
NCORES = 8
BC = B // NCORES          # 128 samples per core
NSTEP = T - 1             # 63
NEVAL = T                 # 64 distinct MLP evaluations
L3 = H * C                # 1088
LIP = 0.909

F32 = mybir.dt.float32
BF16 = mybir.dt.bfloat16
F32R = mybir.dt.float32r
AF = mybir.ActivationFunctionType
ALU = mybir.AluOpType

# dtype of matmul inputs: BF16 (fast) | F32R (tf32-ish) | F32 (exact)
MM_DT = BF16


def _build_program():
    nc = bacc.Bacc("TRN2", target_bir_lowering=False, debug=False)
    mmdt = MM_DT

    # ---- DRAM I/O -------------------------------------------------------
    ys_d = nc.dram_tensor("ys", [BC, T * C], F32, kind="ExternalInput")
    f1hat_d = nc.dram_tensor("f1hat", [H, H], F32, kind="ExternalInput")
    biasa_d = nc.dram_tensor("biasa", [H, NEVAL], F32, kind="ExternalInput")
    f2p_d = nc.dram_tensor("f2p", [H, H], F32, kind="ExternalInput")
    b2f_d = nc.dram_tensor("b2f", [H, 1], F32, kind="ExternalInput")
    f3e_d = nc.dram_tensor("f3e", [H + 1, L3], F32, kind="ExternalInput")
    i1_d = nc.dram_tensor("i1", [C, H], F32, kind="ExternalInput")
    b1i_d = nc.dram_tensor("b1i", [H, 1], F32, kind="ExternalInput")
    i2s_d = nc.dram_tensor("i2s", [H, H], F32, kind="ExternalInput")
    b2i_d = nc.dram_tensor("b2i", [H, 1], F32, kind="ExternalInput")
    i3s_d = nc.dram_tensor("i3s", [H, H], F32, kind="ExternalInput")
    b3ih_d = nc.dram_tensor("b3ih", [H, 1], F32, kind="ExternalInput")
    rwh_d = nc.dram_tensor("rwh", [1, H], F32, kind="ExternalInput")
    out_d = nc.dram_tensor("partial", [1, 1], F32, kind="ExternalOutput")

    # ---- persistent SBUF ------------------------------------------------
    def sb(name, shape, dt=F32):
        return nc.alloc_sbuf_tensor(name, shape, dt).ap()

    ys_t = sb("ys_t", [BC, T * C])
    diff_t = sb("diff_t", [BC, NSTEP * C])
    diff2_t = sb("diff2_t", [BC, NSTEP * C])
    f1hat_t = sb("f1hat_t", [H, H])
    biasa_t = sb("biasa_t", [H, NEVAL])
    f2p_t = sb("f2p_t", [H, H])
    b2f_t = sb("b2f_t", [H, 1])
    f3e_t = sb("f3e_t", [H + 1, L3])
    i1_t = sb("i1_t", [C, H])
    b1i_t = sb("b1i_t", [H, 1])
    i2s_t = sb("i2s_t", [H, H])
    b2i_t = sb("b2i_t", [H, 1])
    i3s_t = sb("i3s_t", [H, H])
    b3ih_t = sb("b3ih_t", [H, 1])
    rwh_t = sb("rwh_t", [1, H])
    ident_t = sb("ident_t", [128, 128])
    identm_t = sb("identm_t", [128, 128], mmdt)
    ones_t = sb("ones_t", [BC, 1])
    # matmul-input copies of the weights, rounded on device
    f1hat_m = sb("f1hat_m", [H, H], mmdt)
    f2p_m = sb("f2p_m", [H, H], mmdt)
    f3e_m = sb("f3e_m", [H + 1, L3], mmdt)
    i1_m = sb("i1_m", [C, H], mmdt)
    i2s_m = sb("i2s_m", [H, H], mmdt)
    i3s_m = sb("i3s_m", [H, H], mmdt)

    with tile.TileContext(nc) as tc:
        with (
            tc.tile_pool(name="psA", bufs=2, space="PSUM") as psA,
            tc.tile_pool(name="psB", bufs=2, space="PSUM") as psB,
            tc.tile_pool(name="sb_a1", bufs=2) as sb_a1,
            tc.tile_pool(name="sb_a2", bufs=2) as sb_a2,
            tc.tile_pool(name="sb_a3", bufs=2) as sb_a3,
            tc.tile_pool(name="sb_T", bufs=2) as sb_T,
            tc.tile_pool(name="sb_acc", bufs=2) as sb_acc,
            tc.tile_pool(name="sb_st", bufs=6) as sb_st,
            tc.tile_pool(name="sb_zf", bufs=2) as sb_zf,
            tc.tile_pool(name="sb_fin", bufs=2) as sb_fin,
        ):
            # ---- loads -------------------------------------------------
            for i in range(4):
                sl = slice(i * (BC // 4), (i + 1) * (BC // 4))
                nc.sync.dma_start(ys_t[sl, :], ys_d[sl, :])
            nc.sync.dma_start(f3e_t[0:33, :], f3e_d[0:33, :])
            nc.sync.dma_start(f3e_t[33:65, :], f3e_d[33:65, :])
            for dst, src in (
                (f1hat_t, f1hat_d), (biasa_t, biasa_d), (f2p_t, f2p_d),
                (b2f_t, b2f_d), (i1_t, i1_d), (b1i_t, b1i_d),
                (i2s_t, i2s_d), (b2i_t, b2i_d), (i3s_t, i3s_d),
                (b3ih_t, b3ih_d), (rwh_t, rwh_d),
            ):
                nc.sync.dma_start(dst[:, :], src[:, :])

            make_identity(nc, ident_t)
            make_identity(nc, identm_t)
            nc.gpsimd.memset(ones_t, 1.0)
            # round matmul inputs to MM_DT once, on device
            nc.scalar.copy(f3e_m[:, :], f3e_t[:, :])
            nc.vector.tensor_copy(f1hat_m[:, :], f1hat_t[:, :])
            nc.vector.tensor_copy(f2p_m[:, :], f2p_t[:, :])
            nc.vector.tensor_copy(i1_m[:, :], i1_t[:, :])
            nc.vector.tensor_copy(i2s_m[:, :], i2s_t[:, :])
            nc.vector.tensor_copy(i3s_m[:, :], i3s_t[:, :])

            # diff[b, t*17+c] = ys[b, (t+1)*17+c] - ys[b, t*17+c]
            nc.vector.tensor_tensor(
                out=diff_t[:, :], in0=ys_t[:, C:], in1=ys_t[:, : NSTEP * C],
                op=ALU.subtract,
            )
            nc.vector.tensor_scalar_mul(diff2_t[:, :], diff_t[:, :], 2.0)

            # ---- init MLP: h0 = sigmoid(mlp3(ys[:, 0])) ----------------
            ps = psA.tile([C, BC], F32, tag="ps_s")
            nc.tensor.transpose(ps[:, :], ys_t[:, 0:C], ident_t[:, :])
            x0 = sb_a1.tile([C, BC], mmdt, tag="a1")
            nc.scalar.copy(x0[:, :], ps[:, :])

            ps1 = psA.tile([H, BC], F32, tag="ps_s")
            nc.tensor.matmul(ps1[:, :], i1_m[:, :], x0[:, :],
                             start=True, stop=True)
            a1 = sb_a1.tile([H, BC], mmdt, tag="a1")
            nc.scalar.activation(a1[:, :], ps1[:, :], AF.Silu, bias=b1i_t[:, :])

            ps2 = psA.tile([H, BC], F32, tag="ps_s")
            nc.tensor.matmul(ps2[:, :], i2s_m[:, :], a1[:, :],
                             start=True, stop=True)
            a2 = sb_a1.tile([H, BC], mmdt, tag="a1")
            nc.scalar.activation(a2[:, :], ps2[:, :], AF.Silu, bias=b2i_t[:, :])

            ps3 = psA.tile([H, BC], F32, tag="ps_s")
            nc.tensor.matmul(ps3[:, :], i3s_m[:, :], a2[:, :],
                             start=True, stop=True)
            # sigmoid(x) = 0.5 + 0.5*tanh(x/2); bias input is 0.5*b3i
            th = sb_a1.tile([H, BC], F32, tag="a1")
            nc.scalar.activation(th[:, :], ps3[:, :], AF.Tanh,
                                 bias=b3ih_t[:, :], scale=0.5)
            h0f = sb_zf.tile([H, BC], mmdt, tag="zf")
            nc.vector.tensor_scalar(h0f[:, :], th[:, :], 0.5, 0.5,
                                    ALU.mult, ALU.add)

            # h0 batch-major (fp32 state)
            psh = psA.tile([BC, H], mmdt, tag="ps_s")
            nc.tensor.transpose(psh[:, :], h0f[:, :], identm_t[0:H, 0:H])
            h0bm = sb_st.tile([BC, H], F32, tag="st")
            nc.scalar.copy(h0bm[:, :], psh[:, :])

            # ---- one func-MLP evaluation + fused einsum/state update ---
            def eval_step(rhs_feat, e, dsrc, dcol, seed):
                """tanh-MLP at time e on state rhs_feat, then
                out = seed + sum_c vf[:, c, :] * dsrc[:, dcol+c]."""
                ps1 = psA.tile([H, BC], F32, tag="ps_s")
                nc.tensor.matmul(ps1[:, :], f1hat_m[:, :], rhs_feat[:, :],
                                 start=True, stop=True)
                a1 = sb_a1.tile([H, BC], mmdt, tag="a1")
                nc.scalar.activation(a1[:, :], ps1[:, :], AF.Silu,
                                     bias=biasa_t[:, e:e + 1])
                ps2 = psA.tile([H, BC], F32, tag="ps_s")
                nc.tensor.matmul(ps2[:, :], f2p_m[:, :], a1[:, :],
                                 start=True, stop=True)
                a2e = sb_a2.tile([H + 1, BC], mmdt, tag="a2e")
                nc.scalar.activation(a2e[0:H, :], ps2[:, :], AF.Silu,
                                     bias=b2f_t[:, :])
                # homogeneous ones row folds the L3 bias into the GEMM
                nc.gpsimd.memset(a2e[H:H + 1, :], 1.0)

                ps3 = psB.tile([BC, 1536], F32, tag="ps_b")
                for n0, nw in ((0, 512), (512, 512), (1024, 64)):
                    nc.tensor.matmul(ps3[:, n0:n0 + nw], a2e[:, :],
                                     f3e_m[:, n0:n0 + nw],
                                     start=True, stop=True)
                a3a = sb_a3.tile([BC, 512], F32, tag="a3a")
                a3b = sb_a3.tile([BC, L3 - 512], F32, tag="a3b")
                nc.scalar.activation(a3a[:, :], ps3[:, 0:512], AF.Tanh)
                nc.scalar.activation(a3b[:, :], ps3[:, 512:L3], AF.Tanh)

                # einsum over c: T = a3 * dx (broadcast over h), then a
                # strided reduce over c and the seed add.
                Tt = sb_T.tile([BC, L3], F32, tag="T")
                dxa = dsrc[:, dcol:dcol + 8][:, :, None] \
                    .broadcast_to([BC, 8, H])
                nc.vector.tensor_tensor(
                    out=Tt[:, 0:512].rearrange("p (c k) -> p c k", c=8),
                    in0=a3a[:, :].rearrange("p (c k) -> p c k", c=8),
                    in1=dxa, op=ALU.mult)
                dxb = dsrc[:, dcol + 8:dcol + C][:, :, None] \
                    .broadcast_to([BC, 9, H])
                nc.vector.tensor_tensor(
                    out=Tt[:, 512:L3].rearrange("p (c k) -> p c k", c=9),
                    in0=a3b[:, :].rearrange("p (c k) -> p c k", c=9),
                    in1=dxb, op=ALU.mult)
                red = sb_acc.tile([BC, H], F32, tag="red")
                nc.vector.tensor_reduce(
                    out=red[:, :],
                    in_=Tt[:, :].rearrange("p (c k) -> p k c", c=C),
                    axis=mybir.AxisListType.X, op=ALU.add)
                dst = sb_st.tile([BC, H], F32, tag="st")
                nc.vector.tensor_tensor(out=dst[:, :], in0=red[:, :],
                                        in1=seed[:, :], op=ALU.add)
                return dst

            def to_feat(z_bm):
                pst = psA.tile([H, BC], F32, tag="ps_s")
                nc.tensor.transpose(pst[:, :], z_bm[:, :], ident_t[:, :])
                z_feat = sb_zf.tile([H, BC], mmdt, tag="zf")
                nc.scalar.copy(z_feat[:, :], pst[:, :])
                return z_feat

            # ---- leapfrog recurrence -----------------------------------
            # eval 0: z_1 = z_0 + f(0, z_0, dX0[0])
            z_prev_bm = h0bm                       # z_0
            z_cur_bm = eval_step(h0f, 0, diff_t, 0, h0bm)   # z_1
            z_cur_feat = to_feat(z_cur_bm)

            for k in range(1, NSTEP):              # k = 1..62
                # z_{k+1} = z_{k-1} + 2 * f(k, z_k, dX0[k])
                z_next = eval_step(z_cur_feat, k, diff2_t, k * C, z_prev_bm)
                z_prev_bm = z_cur_bm
                z_cur_bm = z_next
                z_cur_feat = to_feat(z_cur_bm)

            # eval 63: u_63 = z_62 + z_63 + f(63, z_63, dX0[62])
            seed_u = sb_acc.tile([BC, H], F32, tag="red")
            nc.vector.tensor_tensor(
                out=seed_u[:, :], in0=z_prev_bm[:, :], in1=z_cur_bm[:, :],
                op=ALU.add)
            u_bm = eval_step(z_cur_feat, NSTEP, diff_t, (NSTEP - 1) * C,
                             seed_u)

            # ---- readout: partial = sum_b sum_h u[b,h] * rW[h]/2 -------
            psr = psA.tile([1, H], F32, tag="ps_s")
            nc.tensor.matmul(psr[:, :], ones_t[:, :], u_bm[:, :],
                             start=True, stop=True)
            r0 = sb_fin.tile([1, H], F32)
            nc.scalar.copy(r0[:, :], psr[:, :])
            r1 = sb_fin.tile([1, H], F32)
            nc.vector.tensor_tensor(out=r1[:, :], in0=r0[:, :], in1=rwh_t[:, :],
                                    op=ALU.mult)
            r2 = sb_fin.tile([1, 1], F32)
            nc.vector.tensor_reduce(out=r2[:, :], in_=r1[:, :],
                                    axis=mybir.AxisListType.X, op=ALU.add)
            nc.sync.dma_start(out_d[:, :], r2[:, :])

    nc.finalize()
    return nc


_CACHE = {}


def _get_program():
    if "nc" not in _CACHE:
        _CACHE["nc"] = _build_program()
    return _CACHE["nc"]


def _prep_consts(init_params, func_params, readout_W):
    (i1w, i1b), (i2w, i2b), (i3w, i3b) = [
        (np.asarray(w, np.float32), np.asarray(b, np.float32))
        for w, b in init_params
    ]
    (f1w, f1b), (f2w, f2b), (f3w, f3b) = [
        (np.asarray(w, np.float32), np.asarray(b, np.float32))
        for w, b in func_params
    ]
    rw = np.asarray(readout_W, np.float32).reshape(H)

    f1hat = np.ascontiguousarray(f1w[1:, :])            # [64, 64]
    w_t = f1w[0, :]                                     # [64]
    t_e = np.arange(NEVAL, dtype=np.float32)            # eval k happens at t=k
    biasa = f1b[:, None] + np.outer(w_t, t_e)           # [64, 64]

    f2p = (LIP * f2w).astype(np.float32)

    # permute columns: f3perm[:, c*64+h] = f3w[:, h*17+c]
    f3perm = f3w.reshape(H, H, C).transpose(0, 2, 1).reshape(H, L3)
    b3perm = f3b.reshape(H, C).T.reshape(L3)
    f3e = np.concatenate([LIP * f3perm, b3perm[None, :]], axis=0)  # [65, 1088]

    i2s = (LIP * i2w).astype(np.float32)
    i3s = (LIP * i3w).astype(np.float32)
    b3ih = (0.5 * i3b).astype(np.float32)

    rwh = (0.5 * rw).astype(np.float32)[None, :]        # [1, 64]

    return {
        "f1hat": np.ascontiguousarray(f1hat, np.float32),
        "biasa": np.ascontiguousarray(biasa, np.float32),
        "f2p": np.ascontiguousarray(f2p, np.float32),
        "b2f": np.ascontiguousarray(f2b.reshape(H, 1), np.float32),
        "f3e": np.ascontiguousarray(f3e, np.float32),
        "i1": np.ascontiguousarray(i1w, np.float32),
        "b1i": np.ascontiguousarray(i1b.reshape(H, 1), np.float32),
        "i2s": np.ascontiguousarray(i2s, np.float32),
        "b2i": np.ascontiguousarray(i2b.reshape(H, 1), np.float32),
        "i3s": np.ascontiguousarray(i3s, np.float32),
        "b3ih": np.ascontiguousarray(b3ih.reshape(H, 1), np.float32),
        "rwh": np.ascontiguousarray(rwh, np.float32),
    }


def kernel(ys_coeffs, init_params, func_params, readout_W, readout_b,
           _trace=False, _tmpdir=None):
    ys = np.asarray(ys_coeffs, np.float32)
    assert ys.shape == (B, T, C), ys.shape
    rb = float(np.asarray(readout_b, np.float32).reshape(-1)[0])

    consts = _prep_consts(init_params, func_params, readout_W)
    nc = _get_program()

    in_maps = []
    for cid in range(NCORES):
        m = dict(consts)
        m["ys"] = np.ascontiguousarray(
            ys[cid * BC:(cid + 1) * BC].reshape(BC, T * C))
        in_maps.append(m)

    kw = {}
    if _trace:
        kw = dict(trace=True, tmpdir=_tmpdir)
    res = run_bass_kernel_spmd(nc, in_maps, core_ids=list(range(NCORES)), **kw)
    total = sum(float(r["partial"][0, 0]) for r in res.results)
    out = np.float32(total / B + rb)
    if _trace:
        return np.asarray(out, np.float32), res
    return np.asarray(out, np.float32)


# revision 32
# speedup vs baseline: 1.3712x; 1.0575x over previous
"""Trainium2 Bass kernel for the neural-CDE discriminator.

Reference computation (B=1024, T=64, C=17, HIDDEN=64):
  h0 = init_mlp(ys[:, 0])                    (17 -> 64 -> 64 -> 64, lipswish/sigmoid)
  63 reversible-Heun steps; each step evaluates the func MLP twice:
      vf(t, h) = tanh(mlp([t, h]))           (65 -> 64 -> 64 -> 1088)
      f(t, h, dx) = einsum('bhc,bc->bh', vf.reshape(B, 64, 17), dx)
  score = y_T @ rW + rb; output = mean(score)

Key algebraic restructuring (exact, same arithmetic):
  - f1 of step t and f0 of step t+1 are the same evaluation (same time t+1,
    same state z1, and dX1[t] == dX0[t+1]), so the reference's 126 MLP evals
    reduce to 64 distinct ones.
  - With u = 2y, the reversible-Heun update collapses to a leapfrog:
        z_{k+1} = z_{k-1} + 2 * f(k, z_k, dX0[k])       (k = 1..62)
        z_1     = z_0 + f(0, z_0, dX0[0])
        u_63    = z_62 + z_63 + f(63, z_63, dX0[62])
    Each state update is absorbed into the einsum's seed (dx pre-scaled
    by 2).
  - 0.5 from the Heun average is folded into the readout weights; lipswish's
    0.909 into the next layer's weights; time t enters through a precomputed
    per-eval L1 bias table; the L3 bias rides a homogeneous ones row.

Sharding: pure data parallel, batch 1024 -> 128 per NeuronCore on 8 cores.

Layout per core: L1/L2 activations feature-major [feat, 128] (stationary
weights, no transposes); L3 batch-major via lhsT = activations, rhs = W3 with
columns permuted channel-outer (o' = c*64 + h). Matmul inputs are rounded to
MM_DT (bf16 by default) on device; accumulation stays fp32 in PSUM. The
per-sample contraction over c runs as two broadcast-AP multiplies (pipelined
against the two tanh chunks) + one strided tensor_reduce + a seed add, all
fp32 on the vector engine. One PE transpose + copy per eval turns the new
state batch-major -> feature-major.
"""

import sys

for _p in ("/opt/trn_rl_repo",):
    if _p not in sys.path:
        sys.path.append(_p)

import numpy as np

import concourse.bass as bass
import concourse.mybir as mybir
import concourse.tile as tile
from concourse import bacc
from concourse.bass_utils import run_bass_kernel_spmd
from concourse.masks import make_identity

B = 1024
T = 64
C = 17
H = 64
NCORES = 8
BC = B // NCORES          # 128 samples per core
NSTEP = T - 1             # 63
NEVAL = T                 # 64 distinct MLP evaluations
L3 = H * C                # 1088
LIP = 0.909

F32 = mybir.dt.float32
BF16 = mybir.dt.bfloat16
F32R = mybir.dt.float32r
AF = mybir.ActivationFunctionType
ALU = mybir.AluOpType

# dtypes of matmul inputs per site: BF16 (fast) | F32R (tf32-ish) | F32
Z_DT = F32R     # state z_feat + L1 weights (precision-critical)
A1_DT = F32R    # a1 + L2 weights
A2_DT = BF16    # a2e + L3 weights (large, speed-critical)


def _build_program():
    nc = bacc.Bacc("TRN2", target_bir_lowering=False, debug=False)

    # ---- DRAM I/O -------------------------------------------------------
    ys_d = nc.dram_tensor("ys", [BC, T * C], F32, kind="ExternalInput")
    f1hat_d = nc.dram_tensor("f1hat", [H, H], F32, kind="ExternalInput")
    biasa_d = nc.dram_tensor("biasa", [H, NEVAL], F32, kind="ExternalInput")
    f2p_d = nc.dram_tensor("f2p", [H, H], F32, kind="ExternalInput")
    b2f_d = nc.dram_tensor("b2f", [H, 1], F32, kind="ExternalInput")
    f3e_d = nc.dram_tensor("f3e", [H + 1, L3], F32, kind="ExternalInput")
    i1_d = nc.dram_tensor("i1", [C, H], F32, kind="ExternalInput")
    b1i_d = nc.dram_tensor("b1i", [H, 1], F32, kind="ExternalInput")
    i2s_d = nc.dram_tensor("i2s", [H, H], F32, kind="ExternalInput")
    b2i_d = nc.dram_tensor("b2i", [H, 1], F32, kind="ExternalInput")
    i3s_d = nc.dram_tensor("i3s", [H, H], F32, kind="ExternalInput")
    b3ih_d = nc.dram_tensor("b3ih", [H, 1], F32, kind="ExternalInput")
    rwh_d = nc.dram_tensor("rwh", [1, H], F32, kind="ExternalInput")
    out_d = nc.dram_tensor("partial", [1, 1], F32, kind="ExternalOutput")

    # ---- persistent SBUF ------------------------------------------------
    def sb(name, shape, dt=F32):
        return nc.alloc_sbuf_tensor(name, shape, dt).ap()

    ys_t = sb("ys_t", [BC, T * C])
    diff_t = sb("diff_t", [BC, NSTEP * C])
    diff2_t = sb("diff2_t", [BC, NSTEP * C])
    f1hat_t = sb("f1hat_t", [H, H])
    biasa_t = sb("biasa_t", [H, NEVAL])
    f2p_t = sb("f2p_t", [H, H])
    b2f_t = sb("b2f_t", [H, 1])
    f3e_t = sb("f3e_t", [H + 1, L3])
    i1_t = sb("i1_t", [C, H])
    b1i_t = sb("b1i_t", [H, 1])
    i2s_t = sb("i2s_t", [H, H])
    b2i_t = sb("b2i_t", [H, 1])
    i3s_t = sb("i3s_t", [H, H])
    b3ih_t = sb("b3ih_t", [H, 1])
    rwh_t = sb("rwh_t", [1, H])
    ident_t = sb("ident_t", [128, 128])
    identm_t = sb("identm_t", [128, 128], Z_DT)
    ones_t = sb("ones_t", [BC, 1])
    # matmul-input copies of the weights, rounded on device
    f1hat_m = sb("f1hat_m", [H, H], Z_DT)
    f2p_m = sb("f2p_m", [H, H], A1_DT)
    f3e_m = sb("f3e_m", [H + 1, L3], A2_DT)
    i1_m = sb("i1_m", [C, H], Z_DT)
    i2s_m = sb("i2s_m", [H, H], A1_DT)
    i3s_m = sb("i3s_m", [H, H], A1_DT)
    if A2_DT == F32R:
        onesf_t = sb("onesf_t", [1, BC])
        ones1_r = sb("ones1_r", [1, BC], F32R)
    else:
        ones1_r = None

    with tile.TileContext(nc) as tc:
        with (
            tc.tile_pool(name="psA", bufs=2, space="PSUM") as psA,
            tc.tile_pool(name="psB", bufs=2, space="PSUM") as psB,
            tc.tile_pool(name="sb_a1", bufs=2) as sb_a1,
            tc.tile_pool(name="sb_a2", bufs=2) as sb_a2,
            tc.tile_pool(name="sb_a3", bufs=2) as sb_a3,
            tc.tile_pool(name="sb_T", bufs=2) as sb_T,
            tc.tile_pool(name="sb_acc", bufs=2) as sb_acc,
            tc.tile_pool(name="sb_st", bufs=6) as sb_st,
            tc.tile_pool(name="sb_zf", bufs=2) as sb_zf,
            tc.tile_pool(name="sb_fin", bufs=2) as sb_fin,
        ):
            # ---- loads -------------------------------------------------
            for i in range(4):
                sl = slice(i * (BC // 4), (i + 1) * (BC // 4))
                nc.sync.dma_start(ys_t[sl, :], ys_d[sl, :])
            nc.sync.dma_start(f3e_t[0:33, :], f3e_d[0:33, :])
            nc.sync.dma_start(f3e_t[33:65, :], f3e_d[33:65, :])
            for dst, src in (
                (f1hat_t, f1hat_d), (biasa_t, biasa_d), (f2p_t, f2p_d),
                (b2f_t, b2f_d), (i1_t, i1_d), (b1i_t, b1i_d),
                (i2s_t, i2s_d), (b2i_t, b2i_d), (i3s_t, i3s_d),
                (b3ih_t, b3ih_d), (rwh_t, rwh_d),
            ):
                nc.sync.dma_start(dst[:, :], src[:, :])

            make_identity(nc, ident_t)
            if Z_DT == BF16:
                make_identity(nc, identm_t)
            else:
                nc.vector.tensor_copy(identm_t[:, :], ident_t[:, :])
            nc.gpsimd.memset(ones_t, 1.0)
            if A2_DT == F32R:
                nc.gpsimd.memset(onesf_t, 1.0)
                nc.vector.tensor_copy(ones1_r[:, :], onesf_t[:, :])
            # round matmul inputs to MM_DT once, on device
            nc.scalar.copy(f3e_m[:, :], f3e_t[:, :])
            nc.vector.tensor_copy(f1hat_m[:, :], f1hat_t[:, :])
            nc.vector.tensor_copy(f2p_m[:, :], f2p_t[:, :])
            nc.vector.tensor_copy(i1_m[:, :], i1_t[:, :])
            nc.vector.tensor_copy(i2s_m[:, :], i2s_t[:, :])
            nc.vector.tensor_copy(i3s_m[:, :], i3s_t[:, :])

            # diff[b, t*17+c] = ys[b, (t+1)*17+c] - ys[b, t*17+c]
            nc.vector.tensor_tensor(
                out=diff_t[:, :], in0=ys_t[:, C:], in1=ys_t[:, : NSTEP * C],
                op=ALU.subtract,
            )
            nc.vector.tensor_scalar_mul(diff2_t[:, :], diff_t[:, :], 2.0)

            # ---- init MLP: h0 = sigmoid(mlp3(ys[:, 0])) ----------------
            ps = psA.tile([C, BC], F32, tag="ps_s")
            nc.tensor.transpose(ps[:, :], ys_t[:, 0:C], ident_t[:, :])
            x0 = sb_a1.tile([C, BC], Z_DT, tag="a1")
            nc.scalar.copy(x0[:, :], ps[:, :])

            ps1 = psA.tile([H, BC], F32, tag="ps_s")
            nc.tensor.matmul(ps1[:, :], i1_m[:, :], x0[:, :],
                             start=True, stop=True)
            a1 = sb_a1.tile([H, BC], A1_DT, tag="a1")
            nc.scalar.activation(a1[:, :], ps1[:, :], AF.Silu, bias=b1i_t[:, :])

            ps2 = psA.tile([H, BC], F32, tag="ps_s")
            nc.tensor.matmul(ps2[:, :], i2s_m[:, :], a1[:, :],
                             start=True, stop=True)
            a2 = sb_a1.tile([H, BC], A1_DT, tag="a1")
            nc.scalar.activation(a2[:, :], ps2[:, :], AF.Silu, bias=b2i_t[:, :])

            ps3 = psA.tile([H, BC], F32, tag="ps_s")
            nc.tensor.matmul(ps3[:, :], i3s_m[:, :], a2[:, :],
                             start=True, stop=True)
            # sigmoid(x) = 0.5 + 0.5*tanh(x/2); bias input is 0.5*b3i
            th = sb_a1.tile([H, BC], F32, tag="a1")
            nc.scalar.activation(th[:, :], ps3[:, :], AF.Tanh,
                                 bias=b3ih_t[:, :], scale=0.5)
            h0f = sb_zf.tile([H, BC], Z_DT, tag="zf")
            nc.vector.tensor_scalar(h0f[:, :], th[:, :], 0.5, 0.5,
                                    ALU.mult, ALU.add)

            # h0 batch-major (fp32 state)
            psh = psA.tile([BC, H], Z_DT, tag="ps_s")
            nc.tensor.transpose(psh[:, :], h0f[:, :], identm_t[0:H, 0:H])
            h0bm = sb_st.tile([BC, H], F32, tag="st")
            nc.scalar.copy(h0bm[:, :], psh[:, :])

            # ---- one func-MLP evaluation + fused einsum/state update ---
            def eval_step(rhs_feat, e, dsrc, dcol, seed):
                """tanh-MLP at time e on state rhs_feat, then
                out = seed + sum_c vf[:, c, :] * dsrc[:, dcol+c]."""
                ps1 = psA.tile([H, BC], F32, tag="ps_s")
                nc.tensor.matmul(ps1[:, :], f1hat_m[:, :], rhs_feat[:, :],
                                 start=True, stop=True)
                a1 = sb_a1.tile([H, BC], A1_DT, tag="a1")
                nc.scalar.activation(a1[:, :], ps1[:, :], AF.Silu,
                                     bias=biasa_t[:, e:e + 1])
                ps2 = psA.tile([H, BC], F32, tag="ps_s")
                nc.tensor.matmul(ps2[:, :], f2p_m[:, :], a1[:, :],
                                 start=True, stop=True)
                a2e = sb_a2.tile([H + 1, BC], A2_DT, tag="a2e")
                nc.scalar.activation(a2e[0:H, :], ps2[:, :], AF.Silu,
                                     bias=b2f_t[:, :])
                # homogeneous ones row folds the L3 bias into the GEMM
                if A2_DT == F32R:
                    nc.vector.tensor_copy(a2e[H:H + 1, :], ones1_r[:, :])
                else:
                    nc.gpsimd.memset(a2e[H:H + 1, :], 1.0)

                ps3 = psB.tile([BC, 1536], F32, tag="ps_b")
                for n0, nw in ((0, 512), (512, 512), (1024, 64)):
                    nc.tensor.matmul(ps3[:, n0:n0 + nw], a2e[:, :],
                                     f3e_m[:, n0:n0 + nw],
                                     start=True, stop=True)
                # h-outer layout: a3[b, h*17+c]; tanh in two chunks, then
                # dx-broadcast multiplies over h-aligned ranges so each can
                # start as soon as its tanh chunk lands.
                a3 = sb_a3.tile([BC, L3], F32, tag="a3")
                nc.scalar.activation(a3[:, 0:512], ps3[:, 0:512], AF.Tanh)
                nc.scalar.activation(a3[:, 512:L3], ps3[:, 512:L3], AF.Tanh)

                Tt = sb_T.tile([BC, L3], F32, tag="T")
                dx17 = dsrc[:, dcol:dcol + C]
                nc.vector.tensor_tensor(
                    out=Tt[:, 0:510].rearrange("p (k c) -> p k c", c=C),
                    in0=a3[:, 0:510].rearrange("p (k c) -> p k c", c=C),
                    in1=dx17[:, None, :].broadcast_to([BC, 30, C]),
                    op=ALU.mult)
                nc.vector.tensor_tensor(
                    out=Tt[:, 510:L3].rearrange("p (k c) -> p k c", c=C),
                    in0=a3[:, 510:L3].rearrange("p (k c) -> p k c", c=C),
                    in1=dx17[:, None, :].broadcast_to([BC, 34, C]),
                    op=ALU.mult)
                red = sb_acc.tile([BC, H], F32, tag="red")
                nc.vector.tensor_reduce(
                    out=red[:, :],
                    in_=Tt[:, :].rearrange("p (k c) -> p k c", c=C),
                    axis=mybir.AxisListType.X, op=ALU.add)
                dst = sb_st.tile([BC, H], F32, tag="st")
                nc.vector.tensor_tensor(out=dst[:, :], in0=red[:, :],
                                        in1=seed[:, :], op=ALU.add)
                return dst

            def to_feat(z_bm):
                pst = psA.tile([H, BC], F32, tag="ps_s")
                nc.tensor.transpose(pst[:, :], z_bm[:, :], ident_t[:, :])
                z_feat = sb_zf.tile([H, BC], Z_DT, tag="zf")
                nc.scalar.copy(z_feat[:, :], pst[:, :])
                return z_feat

            # ---- leapfrog recurrence -----------------------------------
            # eval 0: z_1 = z_0 + f(0, z_0, dX0[0])
            z_prev_bm = h0bm                       # z_0
            z_cur_bm = eval_step(h0f, 0, diff_t, 0, h0bm)   # z_1
            z_cur_feat = to_feat(z_cur_bm)

            for k in range(1, NSTEP):              # k = 1..62
                # z_{k+1} = z_{k-1} + 2 * f(k, z_k, dX0[k])
                z_next = eval_step(z_cur_feat, k, diff2_t, k * C, z_prev_bm)
                z_prev_bm = z_cur_bm
                z_cur_bm = z_next
                z_cur_feat = to_feat(z_cur_bm)

            # eval 63: u_63 = z_62 + z_63 + f(63, z_63, dX0[62])
            seed_u = sb_acc.tile([BC, H], F32, tag="red")
            nc.vector.tensor_tensor(
                out=seed_u[:, :], in0=z_prev_bm[:, :], in1=z_cur_bm[:, :],
                op=ALU.add)
            u_bm = eval_step(z_cur_feat, NSTEP, diff_t, (NSTEP - 1) * C,
                             seed_u)

            # ---- readout: partial = sum_b sum_h u[b,h] * rW[h]/2 -------
            psr = psA.tile([1, H], F32, tag="ps_s")
            nc.tensor.matmul(psr[:, :], ones_t[:, :], u_bm[:, :],
                             start=True, stop=True)
            r0 = sb_fin.tile([1, H], F32)
            nc.scalar.copy(r0[:, :], psr[:, :])
            r1 = sb_fin.tile([1, H], F32)
            nc.vector.tensor_tensor(out=r1[:, :], in0=r0[:, :], in1=rwh_t[:, :],
                                    op=ALU.mult)
            r2 = sb_fin.tile([1, 1], F32)
            nc.vector.tensor_reduce(out=r2[:, :], in_=r1[:, :],
                                    axis=mybir.AxisListType.X, op=ALU.add)
            nc.sync.dma_start(out_d[:, :], r2[:, :])

    nc.finalize()
    return nc


_CACHE = {}


def _get_program():
    if "nc" not in _CACHE:
        _CACHE["nc"] = _build_program()
    return _CACHE["nc"]


def _prep_consts(init_params, func_params, readout_W):
    (i1w, i1b), (i2w, i2b), (i3w, i3b) = [
        (np.asarray(w, np.float32), np.asarray(b, np.float32))
        for w, b in init_params
    ]
    (f1w, f1b), (f2w, f2b), (f3w, f3b) = [
        (np.asarray(w, np.float32), np.asarray(b, np.float32))
        for w, b in func_params
    ]
    rw = np.asarray(readout_W, np.float32).reshape(H)

    f1hat = np.ascontiguousarray(f1w[1:, :])            # [64, 64]
    w_t = f1w[0, :]                                     # [64]
    t_e = np.arange(NEVAL, dtype=np.float32)            # eval k happens at t=k
    biasa = f1b[:, None] + np.outer(w_t, t_e)           # [64, 64]

    f2p = (LIP * f2w).astype(np.float32)

    # h-outer layout (original column order o = h*17+c)
    f3e = np.concatenate([LIP * f3w, f3b[None, :]], axis=0)  # [65, 1088]

    i2s = (LIP * i2w).astype(np.float32)
    i3s = (LIP * i3w).astype(np.float32)
    b3ih = (0.5 * i3b).astype(np.float32)

    rwh = (0.5 * rw).astype(np.float32)[None, :]        # [1, 64]

    return {
        "f1hat": np.ascontiguousarray(f1hat, np.float32),
        "biasa": np.ascontiguousarray(biasa, np.float32),
        "f2p": np.ascontiguousarray(f2p, np.float32),
        "b2f": np.ascontiguousarray(f2b.reshape(H, 1), np.float32),
        "f3e": np.ascontiguousarray(f3e, np.float32),
        "i1": np.ascontiguousarray(i1w, np.float32),
        "b1i": np.ascontiguousarray(i1b.reshape(H, 1), np.float32),
        "i2s": np.ascontiguousarray(i2s, np.float32),
        "b2i": np.ascontiguousarray(i2b.reshape(H, 1), np.float32),
        "i3s": np.ascontiguousarray(i3s, np.float32),
        "b3ih": np.ascontiguousarray(b3ih.reshape(H, 1), np.float32),
        "rwh": np.ascontiguousarray(rwh, np.float32),
    }


def kernel(ys_coeffs, init_params, func_params, readout_W, readout_b,
           _trace=False, _tmpdir=None):
    ys = np.asarray(ys_coeffs, np.float32)
    assert ys.shape == (B, T, C), ys.shape
    rb = float(np.asarray(readout_b, np.float32).reshape(-1)[0])

    consts = _prep_consts(init_params, func_params, readout_W)
    nc = _get_program()

    in_maps = []
    for cid in range(NCORES):
        m = dict(consts)
        m["ys"] = np.ascontiguousarray(
            ys[cid * BC:(cid + 1) * BC].reshape(BC, T * C))
        in_maps.append(m)

    kw = {}
    if _trace:
        kw = dict(trace=True, tmpdir=_tmpdir)
    res = run_bass_kernel_spmd(nc, in_maps, core_ids=list(range(NCORES)), **kw)
    total = sum(float(r["partial"][0, 0]) for r in res.results)
    out = np.float32(total / B + rb)
    if _trace:
        return np.asarray(out, np.float32), res
    return np.asarray(out, np.float32)


# revision 36
# speedup vs baseline: 1.3713x; 1.0001x over previous
"""Trainium2 Bass kernel for the neural-CDE discriminator.

Reference computation (B=1024, T=64, C=17, HIDDEN=64):
  h0 = init_mlp(ys[:, 0])                    (17 -> 64 -> 64 -> 64, lipswish/sigmoid)
  63 reversible-Heun steps; each step evaluates the func MLP twice:
      vf(t, h) = tanh(mlp([t, h]))           (65 -> 64 -> 64 -> 1088)
      f(t, h, dx) = einsum('bhc,bc->bh', vf.reshape(B, 64, 17), dx)
  score = y_T @ rW + rb; output = mean(score)

Key algebraic restructuring (exact, same arithmetic):
  - f1 of step t and f0 of step t+1 are the same evaluation (same time t+1,
    same state z1, and dX1[t] == dX0[t+1]), so the reference's 126 MLP evals
    reduce to 64 distinct ones.
  - With u = 2y, the reversible-Heun update collapses to a leapfrog:
        z_{k+1} = z_{k-1} + 2 * f(k, z_k, dX0[k])       (k = 1..62)
        z_1     = z_0 + f(0, z_0, dX0[0])
        u_63    = z_62 + z_63 + f(63, z_63, dX0[62])
    Each state update is absorbed into the einsum's seed (dx pre-scaled
    by 2).
  - 0.5 from the Heun average is folded into the readout weights; lipswish's
    0.909 into the next layer's weights; time t enters through a precomputed
    per-eval L1 bias table; the L3 bias rides a homogeneous ones row.

Sharding: pure data parallel, batch 1024 -> 128 per NeuronCore on 8 cores.

Layout per core: L1/L2 activations feature-major [feat, 128] (stationary
weights, no transposes); L3 batch-major via lhsT = activations, rhs = W3 with
columns permuted channel-outer (o' = c*64 + h). Matmul inputs are rounded to
MM_DT (bf16 by default) on device; accumulation stays fp32 in PSUM. The
per-sample contraction over c runs as two broadcast-AP multiplies (pipelined
against the two tanh chunks) + one strided tensor_reduce + a seed add, all
fp32 on the vector engine. One PE transpose + copy per eval turns the new
state batch-major -> feature-major.
"""

import sys

for _p in ("/opt/trn_rl_repo",):
    if _p not in sys.path:
        sys.path.append(_p)

import numpy as np

import concourse.bass as bass
import concourse.mybir as mybir
import concourse.tile as tile
from concourse import bacc
from concourse.bass_utils import run_bass_kernel_spmd
from concourse.masks import make_identity

B = 1024
T = 64
C = 17
H = 64
NCORES = 8
BC = B // NCORES          # 128 samples per core
NSTEP = T - 1             # 63
NEVAL = T                 # 64 distinct MLP evaluations
L3 = H * C                # 1088
LIP = 0.909

F32 = mybir.dt.float32
BF16 = mybir.dt.bfloat16
F32R = mybir.dt.float32r
AF = mybir.ActivationFunctionType
ALU = mybir.AluOpType

# dtypes of matmul inputs per site: BF16 (fast) | F32R (tf32-ish) | F32
Z_DT = F32R     # state z_feat + L1 weights (precision-critical)
A1_DT = F32R    # a1 + L2 weights
A2_DT = BF16    # a2e + L3 weights (large, speed-critical)
WARM_MM = 3     # dummy matmuls per eval to keep the PE clock-gate warm


def _build_program():
    nc = bacc.Bacc("TRN2", target_bir_lowering=False, debug=False)

    # ---- DRAM I/O -------------------------------------------------------
    ys_d = nc.dram_tensor("ys", [BC, T * C], F32, kind="ExternalInput")
    f1hat_d = nc.dram_tensor("f1hat", [H, H], F32, kind="ExternalInput")
    biasa_d = nc.dram_tensor("biasa", [H, NEVAL], F32, kind="ExternalInput")
    f2p_d = nc.dram_tensor("f2p", [H, H], F32, kind="ExternalInput")
    b2f_d = nc.dram_tensor("b2f", [H, 1], F32, kind="ExternalInput")
    f3e_d = nc.dram_tensor("f3e", [H + 1, L3], F32, kind="ExternalInput")
    i1_d = nc.dram_tensor("i1", [C, H], F32, kind="ExternalInput")
    b1i_d = nc.dram_tensor("b1i", [H, 1], F32, kind="ExternalInput")
    i2s_d = nc.dram_tensor("i2s", [H, H], F32, kind="ExternalInput")
    b2i_d = nc.dram_tensor("b2i", [H, 1], F32, kind="ExternalInput")
    i3s_d = nc.dram_tensor("i3s", [H, H], F32, kind="ExternalInput")
    b3ih_d = nc.dram_tensor("b3ih", [H, 1], F32, kind="ExternalInput")
    rwh_d = nc.dram_tensor("rwh", [1, H], F32, kind="ExternalInput")
    out_d = nc.dram_tensor("partial", [1, 1], F32, kind="ExternalOutput")

    # ---- persistent SBUF ------------------------------------------------
    def sb(name, shape, dt=F32):
        return nc.alloc_sbuf_tensor(name, shape, dt).ap()

    ys_t = sb("ys_t", [BC, T * C])
    diff_t = sb("diff_t", [BC, NSTEP * C])
    diff2_t = sb("diff2_t", [BC, NSTEP * C])
    f1hat_t = sb("f1hat_t", [H, H])
    biasa_t = sb("biasa_t", [H, NEVAL])
    f2p_t = sb("f2p_t", [H, H])
    b2f_t = sb("b2f_t", [H, 1])
    f3e_t = sb("f3e_t", [H + 1, L3])
    i1_t = sb("i1_t", [C, H])
    b1i_t = sb("b1i_t", [H, 1])
    i2s_t = sb("i2s_t", [H, H])
    b2i_t = sb("b2i_t", [H, 1])
    i3s_t = sb("i3s_t", [H, H])
    b3ih_t = sb("b3ih_t", [H, 1])
    rwh_t = sb("rwh_t", [1, H])
    ident_t = sb("ident_t", [128, 128])
    identm_t = sb("identm_t", [128, 128], Z_DT)
    ones_t = sb("ones_t", [BC, 1])
    # matmul-input copies of the weights, rounded on device
    f1hat_m = sb("f1hat_m", [H, H], Z_DT)
    f2p_m = sb("f2p_m", [H, H], A1_DT)
    f3e_m = sb("f3e_m", [H + 1, L3], A2_DT)
    i1_m = sb("i1_m", [C, H], Z_DT)
    i2s_m = sb("i2s_m", [H, H], A1_DT)
    i3s_m = sb("i3s_m", [H, H], A1_DT)
    if A2_DT == F32R:
        onesf_t = sb("onesf_t", [1, BC])
        ones1_r = sb("ones1_r", [1, BC], F32R)
    else:
        ones1_r = None

    with tile.TileContext(nc) as tc:
        with (
            tc.tile_pool(name="psA", bufs=3, space="PSUM") as psA,
            tc.tile_pool(name="psB", bufs=1, space="PSUM") as psB,
            tc.tile_pool(name="psW", bufs=2, space="PSUM") as psW,
            tc.tile_pool(name="sb_a1", bufs=2) as sb_a1,
            tc.tile_pool(name="sb_a2", bufs=2) as sb_a2,
            tc.tile_pool(name="sb_a3", bufs=2) as sb_a3,
            tc.tile_pool(name="sb_T", bufs=2) as sb_T,
            tc.tile_pool(name="sb_acc", bufs=2) as sb_acc,
            tc.tile_pool(name="sb_st", bufs=6) as sb_st,
            tc.tile_pool(name="sb_zf", bufs=2) as sb_zf,
            tc.tile_pool(name="sb_fin", bufs=2) as sb_fin,
        ):
            # ---- loads -------------------------------------------------
            for i in range(4):
                sl = slice(i * (BC // 4), (i + 1) * (BC // 4))
                nc.sync.dma_start(ys_t[sl, :], ys_d[sl, :])
            nc.sync.dma_start(f3e_t[0:33, :], f3e_d[0:33, :])
            nc.sync.dma_start(f3e_t[33:65, :], f3e_d[33:65, :])
            for dst, src in (
                (f1hat_t, f1hat_d), (biasa_t, biasa_d), (f2p_t, f2p_d),
                (b2f_t, b2f_d), (i1_t, i1_d), (b1i_t, b1i_d),
                (i2s_t, i2s_d), (b2i_t, b2i_d), (i3s_t, i3s_d),
                (b3ih_t, b3ih_d), (rwh_t, rwh_d),
            ):
                nc.sync.dma_start(dst[:, :], src[:, :])

            make_identity(nc, ident_t)
            if Z_DT == BF16:
                make_identity(nc, identm_t)
            else:
                nc.vector.tensor_copy(identm_t[:, :], ident_t[:, :])
            nc.gpsimd.memset(ones_t, 1.0)
            if A2_DT == F32R:
                nc.gpsimd.memset(onesf_t, 1.0)
                nc.vector.tensor_copy(ones1_r[:, :], onesf_t[:, :])
            # round matmul inputs to MM_DT once, on device
            nc.scalar.copy(f3e_m[:, :], f3e_t[:, :])
            nc.vector.tensor_copy(f1hat_m[:, :], f1hat_t[:, :])
            nc.vector.tensor_copy(f2p_m[:, :], f2p_t[:, :])
            nc.vector.tensor_copy(i1_m[:, :], i1_t[:, :])
            nc.vector.tensor_copy(i2s_m[:, :], i2s_t[:, :])
            nc.vector.tensor_copy(i3s_m[:, :], i3s_t[:, :])

            # diff[b, t*17+c] = ys[b, (t+1)*17+c] - ys[b, t*17+c]
            nc.vector.tensor_tensor(
                out=diff_t[:, :], in0=ys_t[:, C:], in1=ys_t[:, : NSTEP * C],
                op=ALU.subtract,
            )
            nc.vector.tensor_scalar_mul(diff2_t[:, :], diff_t[:, :], 2.0)

            # ---- init MLP: h0 = sigmoid(mlp3(ys[:, 0])) ----------------
            ps = psA.tile([C, BC], F32, tag="ps_s")
            nc.tensor.transpose(ps[:, :], ys_t[:, 0:C], ident_t[:, :])
            x0 = sb_a1.tile([C, BC], Z_DT, tag="a1")
            nc.scalar.copy(x0[:, :], ps[:, :])

            ps1 = psA.tile([H, BC], F32, tag="ps_s")
            nc.tensor.matmul(ps1[:, :], i1_m[:, :], x0[:, :],
                             start=True, stop=True)
            a1 = sb_a1.tile([H, BC], A1_DT, tag="a1")
            nc.scalar.activation(a1[:, :], ps1[:, :], AF.Silu, bias=b1i_t[:, :])

            ps2 = psA.tile([H, BC], F32, tag="ps_s")
            nc.tensor.matmul(ps2[:, :], i2s_m[:, :], a1[:, :],
                             start=True, stop=True)
            a2 = sb_a1.tile([H, BC], A1_DT, tag="a1")
            nc.scalar.activation(a2[:, :], ps2[:, :], AF.Silu, bias=b2i_t[:, :])

            ps3 = psA.tile([H, BC], F32, tag="ps_s")
            nc.tensor.matmul(ps3[:, :], i3s_m[:, :], a2[:, :],
                             start=True, stop=True)
            # sigmoid(x) = 0.5 + 0.5*tanh(x/2); bias input is 0.5*b3i
            th = sb_a1.tile([H, BC], F32, tag="a1")
            nc.scalar.activation(th[:, :], ps3[:, :], AF.Tanh,
                                 bias=b3ih_t[:, :], scale=0.5)
            h0f = sb_zf.tile([H, BC], Z_DT, tag="zf")
            nc.vector.tensor_scalar(h0f[:, :], th[:, :], 0.5, 0.5,
                                    ALU.mult, ALU.add)

            # h0 batch-major (fp32 state)
            psh = psA.tile([BC, H], Z_DT, tag="ps_s")
            nc.tensor.transpose(psh[:, :], h0f[:, :], identm_t[0:H, 0:H])
            h0bm = sb_st.tile([BC, H], F32, tag="st")
            nc.scalar.copy(h0bm[:, :], psh[:, :])

            # ---- one func-MLP evaluation + fused einsum/state update ---
            def eval_step(rhs_feat, e, dsrc, dcol, seed):
                """tanh-MLP at time e on state rhs_feat, then
                out = seed + sum_c vf[:, c, :] * dsrc[:, dcol+c]."""
                ps1 = psA.tile([H, BC], F32, tag="ps_s")
                nc.tensor.matmul(ps1[:, :], f1hat_m[:, :], rhs_feat[:, :],
                                 start=True, stop=True)
                a1 = sb_a1.tile([H, BC], A1_DT, tag="a1")
                nc.scalar.activation(a1[:, :], ps1[:, :], AF.Silu,
                                     bias=biasa_t[:, e:e + 1])
                ps2 = psA.tile([H, BC], F32, tag="ps_s")
                nc.tensor.matmul(ps2[:, :], f2p_m[:, :], a1[:, :],
                                 start=True, stop=True)
                a2e = sb_a2.tile([H + 1, BC], A2_DT, tag="a2e")
                nc.scalar.activation(a2e[0:H, :], ps2[:, :], AF.Silu,
                                     bias=b2f_t[:, :])
                # homogeneous ones row folds the L3 bias into the GEMM
                if A2_DT == F32R:
                    nc.vector.tensor_copy(a2e[H:H + 1, :], ones1_r[:, :])
                else:
                    nc.gpsimd.memset(a2e[H:H + 1, :], 1.0)

                ps3 = psB.tile([BC, 1536], F32, tag="ps_b")
                for n0, nw in ((0, 512), (512, 512), (1024, 64)):
                    nc.tensor.matmul(ps3[:, n0:n0 + nw], a2e[:, :],
                                     f3e_m[:, n0:n0 + nw],
                                     start=True, stop=True)
                # h-outer layout: a3[b, h*17+c]; tanh in two chunks, then
                # dx-broadcast multiplies over h-aligned ranges so each can
                # start as soon as its tanh chunk lands.
                a3 = sb_a3.tile([BC, L3], F32, tag="a3")
                nc.scalar.activation(a3[:, 0:512], ps3[:, 0:512], AF.Tanh)
                nc.scalar.activation(a3[:, 512:L3], ps3[:, 512:L3], AF.Tanh)

                Tt = sb_T.tile([BC, L3], F32, tag="T")
                dx17 = dsrc[:, dcol:dcol + C]
                nc.vector.tensor_tensor(
                    out=Tt[:, 0:510].rearrange("p (k c) -> p k c", c=C),
                    in0=a3[:, 0:510].rearrange("p (k c) -> p k c", c=C),
                    in1=dx17[:, None, :].broadcast_to([BC, 30, C]),
                    op=ALU.mult)
                nc.vector.tensor_tensor(
                    out=Tt[:, 510:L3].rearrange("p (k c) -> p k c", c=C),
                    in0=a3[:, 510:L3].rearrange("p (k c) -> p k c", c=C),
                    in1=dx17[:, None, :].broadcast_to([BC, 34, C]),
                    op=ALU.mult)
                red = sb_acc.tile([BC, H], F32, tag="red")
                nc.vector.tensor_reduce(
                    out=red[:, :],
                    in_=Tt[:, :].rearrange("p (k c) -> p k c", c=C),
                    axis=mybir.AxisListType.X, op=ALU.add)
                dst = sb_st.tile([BC, H], F32, tag="st")
                nc.vector.tensor_tensor(out=dst[:, :], in0=red[:, :],
                                        in1=seed[:, :], op=ALU.add)

                # PE keep-warm: tiny matmuls anchored to this eval's
                # tensors so they land in the PE-idle tanh/einsum window.
                if WARM_MM:
                    anchors = [a3, Tt, red][:WARM_MM]
                    for anc in anchors:
                        w = psW.tile([H, H], F32, tag="warm")
                        nc.tensor.matmul(w[:, :], f3e_m[0:H, 0:H],
                                         anc[0:H, 0:32].bitcast(BF16),
                                         start=True, stop=True)
                return dst

            def to_feat(z_bm):
                pst = psA.tile([H, BC], F32, tag="ps_s")
                nc.tensor.transpose(pst[:, :], z_bm[:, :], ident_t[:, :])
                z_feat = sb_zf.tile([H, BC], Z_DT, tag="zf")
                nc.scalar.copy(z_feat[:, :], pst[:, :])
                return z_feat

            # ---- leapfrog recurrence -----------------------------------
            # eval 0: z_1 = z_0 + f(0, z_0, dX0[0])
            z_prev_bm = h0bm                       # z_0
            z_cur_bm = eval_step(h0f, 0, diff_t, 0, h0bm)   # z_1
            z_cur_feat = to_feat(z_cur_bm)

            for k in range(1, NSTEP):              # k = 1..62
                # z_{k+1} = z_{k-1} + 2 * f(k, z_k, dX0[k])
                z_next = eval_step(z_cur_feat, k, diff2_t, k * C, z_prev_bm)
                z_prev_bm = z_cur_bm
                z_cur_bm = z_next
                z_cur_feat = to_feat(z_cur_bm)

            # eval 63: u_63 = z_62 + z_63 + f(63, z_63, dX0[62])
            seed_u = sb_acc.tile([BC, H], F32, tag="red")
            nc.vector.tensor_tensor(
                out=seed_u[:, :], in0=z_prev_bm[:, :], in1=z_cur_bm[:, :],
                op=ALU.add)
            u_bm = eval_step(z_cur_feat, NSTEP, diff_t, (NSTEP - 1) * C,
                             seed_u)

            # ---- readout: partial = sum_b sum_h u[b,h] * rW[h]/2 -------
            psr = psA.tile([1, H], F32, tag="ps_s")
            nc.tensor.matmul(psr[:, :], ones_t[:, :], u_bm[:, :],
                             start=True, stop=True)
            r0 = sb_fin.tile([1, H], F32)
            nc.scalar.copy(r0[:, :], psr[:, :])
            r1 = sb_fin.tile([1, H], F32)
            nc.vector.tensor_tensor(out=r1[:, :], in0=r0[:, :], in1=rwh_t[:, :],
                                    op=ALU.mult)
            r2 = sb_fin.tile([1, 1], F32)
            nc.vector.tensor_reduce(out=r2[:, :], in_=r1[:, :],
                                    axis=mybir.AxisListType.X, op=ALU.add)
            nc.sync.dma_start(out_d[:, :], r2[:, :])

    nc.finalize()
    return nc


_CACHE = {}


def _get_program():
    if "nc" not in _CACHE:
        _CACHE["nc"] = _build_program()
    return _CACHE["nc"]


def _prep_consts(init_params, func_params, readout_W):
    (i1w, i1b), (i2w, i2b), (i3w, i3b) = [
        (np.asarray(w, np.float32), np.asarray(b, np.float32))
        for w, b in init_params
    ]
    (f1w, f1b), (f2w, f2b), (f3w, f3b) = [
        (np.asarray(w, np.float32), np.asarray(b, np.float32))
        for w, b in func_params
    ]
    rw = np.asarray(readout_W, np.float32).reshape(H)

    f1hat = np.ascontiguousarray(f1w[1:, :])            # [64, 64]
    w_t = f1w[0, :]                                     # [64]
    t_e = np.arange(NEVAL, dtype=np.float32)            # eval k happens at t=k
    biasa = f1b[:, None] + np.outer(w_t, t_e)           # [64, 64]

    f2p = (LIP * f2w).astype(np.float32)

    # h-outer layout (original column order o = h*17+c)
    f3e = np.concatenate([LIP * f3w, f3b[None, :]], axis=0)  # [65, 1088]

    i2s = (LIP * i2w).astype(np.float32)
    i3s = (LIP * i3w).astype(np.float32)
    b3ih = (0.5 * i3b).astype(np.float32)

    rwh = (0.5 * rw).astype(np.float32)[None, :]        # [1, 64]

    return {
        "f1hat": np.ascontiguousarray(f1hat, np.float32),
        "biasa": np.ascontiguousarray(biasa, np.float32),
        "f2p": np.ascontiguousarray(f2p, np.float32),
        "b2f": np.ascontiguousarray(f2b.reshape(H, 1), np.float32),
        "f3e": np.ascontiguousarray(f3e, np.float32),
        "i1": np.ascontiguousarray(i1w, np.float32),
        "b1i": np.ascontiguousarray(i1b.reshape(H, 1), np.float32),
        "i2s": np.ascontiguousarray(i2s, np.float32),
        "b2i": np.ascontiguousarray(i2b.reshape(H, 1), np.float32),
        "i3s": np.ascontiguousarray(i3s, np.float32),
        "b3ih": np.ascontiguousarray(b3ih.reshape(H, 1), np.float32),
        "rwh": np.ascontiguousarray(rwh, np.float32),
    }


def kernel(ys_coeffs, init_params, func_params, readout_W, readout_b,
           _trace=False, _tmpdir=None):
    ys = np.asarray(ys_coeffs, np.float32)
    assert ys.shape == (B, T, C), ys.shape
    rb = float(np.asarray(readout_b, np.float32).reshape(-1)[0])

    consts = _prep_consts(init_params, func_params, readout_W)
    nc = _get_program()

    in_maps = []
    for cid in range(NCORES):
        m = dict(consts)
        m["ys"] = np.ascontiguousarray(
            ys[cid * BC:(cid + 1) * BC].reshape(BC, T * C))
        in_maps.append(m)

    kw = {}
    if _trace:
        kw = dict(trace=True, tmpdir=_tmpdir)
    res = run_bass_kernel_spmd(nc, in_maps, core_ids=list(range(NCORES)), **kw)
    total = sum(float(r["partial"][0, 0]) for r in res.results)
    out = np.float32(total / B + rb)
    if _trace:
        return np.asarray(out, np.float32), res
    return np.asarray(out, np.float32)


# revision 41
# speedup vs baseline: 1.5254x; 1.1124x over previous
"""Trainium2 Bass kernel for the neural-CDE discriminator.

Reference computation (B=1024, T=64, C=17, HIDDEN=64):
  h0 = init_mlp(ys[:, 0])                    (17 -> 64 -> 64 -> 64, lipswish/sigmoid)
  63 reversible-Heun steps; each step evaluates the func MLP twice:
      vf(t, h) = tanh(mlp([t, h]))           (65 -> 64 -> 64 -> 1088)
      f(t, h, dx) = einsum('bhc,bc->bh', vf.reshape(B, 64, 17), dx)
  score = y_T @ rW + rb; output = mean(score)

Key algebraic restructuring (exact, same arithmetic):
  - f1 of step t and f0 of step t+1 are the same evaluation (same time t+1,
    same state z1, and dX1[t] == dX0[t+1]), so the reference's 126 MLP evals
    reduce to 64 distinct ones.
  - With u = 2y, the reversible-Heun update collapses to a leapfrog:
        z_{k+1} = z_{k-1} + 2 * f(k, z_k, dX0[k])       (k = 1..62)
        z_1     = z_0 + f(0, z_0, dX0[0])
        u_63    = z_62 + z_63 + f(63, z_63, dX0[62])
    Each state update is absorbed into the einsum's seed (dx pre-scaled
    by 2).
  - 0.5 from the Heun average is folded into the readout weights; lipswish's
    0.909 into the next layer's weights; time t enters through a precomputed
    per-eval L1 bias table; the L3 bias rides a homogeneous ones row.

Sharding: pure data parallel, batch 1024 -> 128 per NeuronCore on 8 cores.

Layout per core: L1/L2 activations feature-major [feat, 128] (stationary
weights, no transposes); L3 batch-major via lhsT = activations, rhs = W3 with
columns permuted channel-outer (o' = c*64 + h). Matmul inputs are rounded to
MM_DT (bf16 by default) on device; accumulation stays fp32 in PSUM. The
per-sample contraction over c runs as two broadcast-AP multiplies (pipelined
against the two tanh chunks) + one strided tensor_reduce + a seed add, all
fp32 on the vector engine. One PE transpose + copy per eval turns the new
state batch-major -> feature-major.
"""

import sys

for _p in ("/opt/trn_rl_repo",):
    if _p not in sys.path:
        sys.path.append(_p)

import numpy as np

import concourse.bass as bass
import concourse.mybir as mybir
import concourse.tile as tile
from concourse import bacc
from concourse.bass_utils import run_bass_kernel_spmd
from concourse.masks import make_identity

B = 1024
T = 64
C = 17
H = 64
NCORES = 8
BC = B // NCORES          # 128 samples per core
NSTEP = T - 1             # 63
NEVAL = T                 # 64 distinct MLP evaluations
L3 = H * C                # 1088
LIP = 0.909

F32 = mybir.dt.float32
BF16 = mybir.dt.bfloat16
F32R = mybir.dt.float32r
AF = mybir.ActivationFunctionType
ALU = mybir.AluOpType

# dtypes of matmul inputs per site: BF16 (fast) | F32R (tf32-ish) | F32
Z_DT = F32R     # state/red feat + L1 weights (precision-critical)
A1_DT = F32R    # a1 + L2 weights
A2_DT = BF16    # a2e + L3 weights (large, speed-critical)


def _build_program():
    nc = bacc.Bacc("TRN2", target_bir_lowering=False, debug=False)

    # ---- DRAM I/O -------------------------------------------------------
    ys_d = nc.dram_tensor("ys", [BC, T * C], F32, kind="ExternalInput")
    f1hat_d = nc.dram_tensor("f1hat", [H, H], F32, kind="ExternalInput")
    biasa_d = nc.dram_tensor("biasa", [H, NEVAL], F32, kind="ExternalInput")
    f2p_d = nc.dram_tensor("f2p", [H, H], F32, kind="ExternalInput")
    b2f_d = nc.dram_tensor("b2f", [H, 1], F32, kind="ExternalInput")
    f3e_d = nc.dram_tensor("f3e", [H + 1, L3], F32, kind="ExternalInput")
    i1_d = nc.dram_tensor("i1", [C, H], F32, kind="ExternalInput")
    b1i_d = nc.dram_tensor("b1i", [H, 1], F32, kind="ExternalInput")
    i2s_d = nc.dram_tensor("i2s", [H, H], F32, kind="ExternalInput")
    b2i_d = nc.dram_tensor("b2i", [H, 1], F32, kind="ExternalInput")
    i3s_d = nc.dram_tensor("i3s", [H, H], F32, kind="ExternalInput")
    b3ih_d = nc.dram_tensor("b3ih", [H, 1], F32, kind="ExternalInput")
    rwh_d = nc.dram_tensor("rwh", [1, H], F32, kind="ExternalInput")
    out_d = nc.dram_tensor("partial", [1, 1], F32, kind="ExternalOutput")

    # ---- persistent SBUF ------------------------------------------------
    def sb(name, shape, dt=F32):
        return nc.alloc_sbuf_tensor(name, shape, dt).ap()

    ys_t = sb("ys_t", [BC, T * C])
    diff_t = sb("diff_t", [BC, NSTEP * C])
    diff2_t = sb("diff2_t", [BC, NSTEP * C])
    f1hat_t = sb("f1hat_t", [H, H])
    biasa_t = sb("biasa_t", [H, NEVAL])
    f2p_t = sb("f2p_t", [H, H])
    b2f_t = sb("b2f_t", [H, 1])
    f3e_t = sb("f3e_t", [H + 1, L3])
    i1_t = sb("i1_t", [C, H])
    b1i_t = sb("b1i_t", [H, 1])
    i2s_t = sb("i2s_t", [H, H])
    b2i_t = sb("b2i_t", [H, 1])
    i3s_t = sb("i3s_t", [H, H])
    b3ih_t = sb("b3ih_t", [H, 1])
    rwh_t = sb("rwh_t", [1, H])
    ident_t = sb("ident_t", [128, 128])
    identm_t = sb("identm_t", [128, 128], Z_DT)
    ones_t = sb("ones_t", [BC, 1])
    # matmul-input copies of the weights, rounded on device
    f1hat_m = sb("f1hat_m", [H, H], Z_DT)
    f2p_m = sb("f2p_m", [H, H], A1_DT)
    f3e_m = sb("f3e_m", [H + 1, L3], A2_DT)
    i1_m = sb("i1_m", [C, H], Z_DT)
    i2s_m = sb("i2s_m", [H, H], A1_DT)
    i3s_m = sb("i3s_m", [H, H], A1_DT)
    if A2_DT == F32R:
        onesf_t = sb("onesf_t", [1, BC])
        ones1_r = sb("ones1_r", [1, BC], F32R)
    else:
        ones1_r = None

    # persistent ping-pong PSUM accumulators holding F1^T z_e:
    # F1^T z_{e+1} = F1^T z_{e-1} + F1^T red_e  (leapfrog linearity)
    ps_pp = [nc.alloc_psum_tensor("ps_pp0", [H, BC], F32).ap(),
             nc.alloc_psum_tensor("ps_pp1", [H, BC], F32).ap()]

    with tile.TileContext(nc) as tc:
        with (
            tc.tile_pool(name="psA", bufs=2, space="PSUM") as psA,
            tc.tile_pool(name="psB", bufs=1, space="PSUM") as psB,
            tc.tile_pool(name="sb_a1", bufs=2) as sb_a1,
            tc.tile_pool(name="sb_a2", bufs=2) as sb_a2,
            tc.tile_pool(name="sb_a3", bufs=2) as sb_a3,
            tc.tile_pool(name="sb_T", bufs=2) as sb_T,
            tc.tile_pool(name="sb_acc", bufs=2) as sb_acc,
            tc.tile_pool(name="sb_st", bufs=6) as sb_st,
            tc.tile_pool(name="sb_zf", bufs=2) as sb_zf,
            tc.tile_pool(name="sb_fin", bufs=2) as sb_fin,
        ):
            # ---- loads -------------------------------------------------
            for i in range(4):
                sl = slice(i * (BC // 4), (i + 1) * (BC // 4))
                nc.sync.dma_start(ys_t[sl, :], ys_d[sl, :])
            nc.sync.dma_start(f3e_t[0:33, :], f3e_d[0:33, :])
            nc.sync.dma_start(f3e_t[33:65, :], f3e_d[33:65, :])
            for dst, src in (
                (f1hat_t, f1hat_d), (biasa_t, biasa_d), (f2p_t, f2p_d),
                (b2f_t, b2f_d), (i1_t, i1_d), (b1i_t, b1i_d),
                (i2s_t, i2s_d), (b2i_t, b2i_d), (i3s_t, i3s_d),
                (b3ih_t, b3ih_d), (rwh_t, rwh_d),
            ):
                nc.sync.dma_start(dst[:, :], src[:, :])

            make_identity(nc, ident_t)
            if Z_DT == BF16:
                make_identity(nc, identm_t)
            else:
                nc.vector.tensor_copy(identm_t[:, :], ident_t[:, :])
            nc.gpsimd.memset(ones_t, 1.0)
            if A2_DT == F32R:
                nc.gpsimd.memset(onesf_t, 1.0)
                nc.vector.tensor_copy(ones1_r[:, :], onesf_t[:, :])
            # round matmul inputs to MM_DT once, on device
            nc.scalar.copy(f3e_m[:, :], f3e_t[:, :])
            nc.vector.tensor_copy(f1hat_m[:, :], f1hat_t[:, :])
            nc.vector.tensor_copy(f2p_m[:, :], f2p_t[:, :])
            nc.vector.tensor_copy(i1_m[:, :], i1_t[:, :])
            nc.vector.tensor_copy(i2s_m[:, :], i2s_t[:, :])
            nc.vector.tensor_copy(i3s_m[:, :], i3s_t[:, :])

            # diff[b, t*17+c] = ys[b, (t+1)*17+c] - ys[b, t*17+c]
            nc.vector.tensor_tensor(
                out=diff_t[:, :], in0=ys_t[:, C:], in1=ys_t[:, : NSTEP * C],
                op=ALU.subtract,
            )
            nc.vector.tensor_scalar_mul(diff2_t[:, :], diff_t[:, :], 2.0)

            # ---- init MLP: h0 = sigmoid(mlp3(ys[:, 0])) ----------------
            ps = psA.tile([C, BC], F32, tag="ps_s")
            nc.tensor.transpose(ps[:, :], ys_t[:, 0:C], ident_t[:, :])
            x0 = sb_a1.tile([C, BC], Z_DT, tag="a1")
            nc.scalar.copy(x0[:, :], ps[:, :])

            ps1 = psA.tile([H, BC], F32, tag="ps_s")
            nc.tensor.matmul(ps1[:, :], i1_m[:, :], x0[:, :],
                             start=True, stop=True)
            a1 = sb_a1.tile([H, BC], A1_DT, tag="a1")
            nc.scalar.activation(a1[:, :], ps1[:, :], AF.Silu, bias=b1i_t[:, :])

            ps2 = psA.tile([H, BC], F32, tag="ps_s")
            nc.tensor.matmul(ps2[:, :], i2s_m[:, :], a1[:, :],
                             start=True, stop=True)
            a2 = sb_a1.tile([H, BC], A1_DT, tag="a1")
            nc.scalar.activation(a2[:, :], ps2[:, :], AF.Silu, bias=b2i_t[:, :])

            ps3 = psA.tile([H, BC], F32, tag="ps_s")
            nc.tensor.matmul(ps3[:, :], i3s_m[:, :], a2[:, :],
                             start=True, stop=True)
            # sigmoid(x) = 0.5 + 0.5*tanh(x/2); bias input is 0.5*b3i
            th = sb_a1.tile([H, BC], F32, tag="a1")
            nc.scalar.activation(th[:, :], ps3[:, :], AF.Tanh,
                                 bias=b3ih_t[:, :], scale=0.5)
            h0f = sb_zf.tile([H, BC], Z_DT, tag="zf")
            nc.vector.tensor_scalar(h0f[:, :], th[:, :], 0.5, 0.5,
                                    ALU.mult, ALU.add)

            # h0 batch-major (fp32 state)
            psh = psA.tile([BC, H], Z_DT, tag="ps_s")
            nc.tensor.transpose(psh[:, :], h0f[:, :], identm_t[0:H, 0:H])
            h0bm = sb_st.tile([BC, H], F32, tag="st")
            nc.scalar.copy(h0bm[:, :], psh[:, :])

            # ---- one func-MLP evaluation + fused einsum/state update ---
            def eval_step(e, dsrc, dcol, seed):
                """tanh-MLP at time e (L1 comes pre-accumulated in
                ps_pp[e%2]), then red = einsum(vf, dx); feeds
                ps_pp[(e+1)%2] += F1^T red and returns seed + red."""
                ps1 = ps_pp[e % 2]
                a1 = sb_a1.tile([H, BC], A1_DT, tag="a1")
                nc.scalar.activation(a1[:, :], ps1[:, :], AF.Silu,
                                     bias=biasa_t[:, e:e + 1])
                ps2 = psA.tile([H, BC], F32, tag="ps_s")
                nc.tensor.matmul(ps2[:, :], f2p_m[:, :], a1[:, :],
                                 start=True, stop=True)
                a2e = sb_a2.tile([H + 1, BC], A2_DT, tag="a2e")
                nc.scalar.activation(a2e[0:H, :], ps2[:, :], AF.Silu,
                                     bias=b2f_t[:, :])
                # homogeneous ones row folds the L3 bias into the GEMM
                if A2_DT == F32R:
                    nc.vector.tensor_copy(a2e[H:H + 1, :], ones1_r[:, :])
                else:
                    nc.gpsimd.memset(a2e[H:H + 1, :], 1.0)

                # separate PSUM tiles per chunk so each tanh only waits
                # for its own matmul
                pc0 = psB.tile([BC, 512], F32, tag="b0")
                pc1 = psB.tile([BC, 512], F32, tag="b1")
                pc2 = psB.tile([BC, 64], F32, tag="b2")
                pcs = [pc0, pc1, pc2]
                for pc, (n0, nw) in zip(pcs, ((0, 512), (512, 512),
                                              (1024, 64))):
                    nc.tensor.matmul(pc[:, 0:nw], a2e[:, :],
                                     f3e_m[:, n0:n0 + nw],
                                     start=True, stop=True)
                # h-outer layout: a3[b, h*17+c]
                a3 = sb_a3.tile([BC, L3], F32, tag="a3")
                nc.scalar.activation(a3[:, 0:512], pcs[0][:, :], AF.Tanh)
                nc.scalar.activation(a3[:, 512:1024], pcs[1][:, :], AF.Tanh)
                nc.scalar.activation(a3[:, 1024:L3], pcs[2][:, 0:64], AF.Tanh)

                Tt = sb_T.tile([BC, L3], F32, tag="T")
                dx17 = dsrc[:, dcol:dcol + C]
                nc.vector.tensor_tensor(
                    out=Tt[:, 0:510].rearrange("p (k c) -> p k c", c=C),
                    in0=a3[:, 0:510].rearrange("p (k c) -> p k c", c=C),
                    in1=dx17[:, None, :].broadcast_to([BC, 30, C]),
                    op=ALU.mult)
                nc.vector.tensor_tensor(
                    out=Tt[:, 510:L3].rearrange("p (k c) -> p k c", c=C),
                    in0=a3[:, 510:L3].rearrange("p (k c) -> p k c", c=C),
                    in1=dx17[:, None, :].broadcast_to([BC, 34, C]),
                    op=ALU.mult)
                red = sb_acc.tile([BC, H], F32, tag="red")
                nc.vector.tensor_reduce(
                    out=red[:, :],
                    in_=Tt[:, :].rearrange("p (k c) -> p k c", c=C),
                    axis=mybir.AxisListType.X, op=ALU.add)

                if e < NSTEP:
                    # next eval's L1: ps_pp[(e+1)%2] += F1^T red
                    pst = psA.tile([H, BC], F32, tag="ps_s")
                    nc.tensor.transpose(pst[:, :], red[:, :], ident_t[:, :])
                    red_feat = sb_zf.tile([H, BC], Z_DT, tag="zf")
                    nc.vector.tensor_copy(red_feat[:, :], pst[:, :])
                    nc.tensor.matmul(ps_pp[(e + 1) % 2], f1hat_m[:, :],
                                     red_feat[:, :], start=False, stop=True,
                                     skip_group_check=True)

                dst = sb_st.tile([BC, H], F32, tag="st")
                nc.vector.tensor_tensor(out=dst[:, :], in0=red[:, :],
                                        in1=seed[:, :], op=ALU.add)
                return dst

            # ---- leapfrog recurrence -----------------------------------
            # ps_pp[0] = F1^T z_0, ps_pp[1] = F1^T z_0 (z_{-1} := z_0)
            nc.tensor.matmul(ps_pp[0], f1hat_m[:, :], h0f[:, :],
                             start=True, stop=True)
            nc.tensor.matmul(ps_pp[1], f1hat_m[:, :], h0f[:, :],
                             start=True, stop=True)

            z_prev_bm = h0bm                       # z_0
            z_cur_bm = eval_step(0, diff_t, 0, h0bm)        # z_1

            for k in range(1, NSTEP):              # k = 1..62
                # z_{k+1} = z_{k-1} + 2 * f(k, z_k, dX0[k])
                z_next = eval_step(k, diff2_t, k * C, z_prev_bm)
                z_prev_bm = z_cur_bm
                z_cur_bm = z_next

            # eval 63: u_63 = z_62 + z_63 + f(63, z_63, dX0[62])
            seed_u = sb_acc.tile([BC, H], F32, tag="red")
            nc.vector.tensor_tensor(
                out=seed_u[:, :], in0=z_prev_bm[:, :], in1=z_cur_bm[:, :],
                op=ALU.add)
            u_bm = eval_step(NSTEP, diff_t, (NSTEP - 1) * C, seed_u)

            # ---- readout: partial = sum_b sum_h u[b,h] * rW[h]/2 -------
            psr = psA.tile([1, H], F32, tag="ps_s")
            nc.tensor.matmul(psr[:, :], ones_t[:, :], u_bm[:, :],
                             start=True, stop=True)
            r0 = sb_fin.tile([1, H], F32)
            nc.scalar.copy(r0[:, :], psr[:, :])
            r1 = sb_fin.tile([1, H], F32)
            nc.vector.tensor_tensor(out=r1[:, :], in0=r0[:, :], in1=rwh_t[:, :],
                                    op=ALU.mult)
            r2 = sb_fin.tile([1, 1], F32)
            nc.vector.tensor_reduce(out=r2[:, :], in_=r1[:, :],
                                    axis=mybir.AxisListType.X, op=ALU.add)
            nc.sync.dma_start(out_d[:, :], r2[:, :])

    nc.finalize()
    return nc


_CACHE = {}


def _get_program():
    if "nc" not in _CACHE:
        _CACHE["nc"] = _build_program()
    return _CACHE["nc"]


def _prep_consts(init_params, func_params, readout_W):
    (i1w, i1b), (i2w, i2b), (i3w, i3b) = [
        (np.asarray(w, np.float32), np.asarray(b, np.float32))
        for w, b in init_params
    ]
    (f1w, f1b), (f2w, f2b), (f3w, f3b) = [
        (np.asarray(w, np.float32), np.asarray(b, np.float32))
        for w, b in func_params
    ]
    rw = np.asarray(readout_W, np.float32).reshape(H)

    f1hat = np.ascontiguousarray(f1w[1:, :])            # [64, 64]
    w_t = f1w[0, :]                                     # [64]
    t_e = np.arange(NEVAL, dtype=np.float32)            # eval k happens at t=k
    biasa = f1b[:, None] + np.outer(w_t, t_e)           # [64, 64]

    f2p = (LIP * f2w).astype(np.float32)

    # h-outer layout (original column order o = h*17+c)
    f3e = np.concatenate([LIP * f3w, f3b[None, :]], axis=0)  # [65, 1088]

    i2s = (LIP * i2w).astype(np.float32)
    i3s = (LIP * i3w).astype(np.float32)
    b3ih = (0.5 * i3b).astype(np.float32)

    rwh = (0.5 * rw).astype(np.float32)[None, :]        # [1, 64]

    return {
        "f1hat": np.ascontiguousarray(f1hat, np.float32),
        "biasa": np.ascontiguousarray(biasa, np.float32),
        "f2p": np.ascontiguousarray(f2p, np.float32),
        "b2f": np.ascontiguousarray(f2b.reshape(H, 1), np.float32),
        "f3e": np.ascontiguousarray(f3e, np.float32),
        "i1": np.ascontiguousarray(i1w, np.float32),
        "b1i": np.ascontiguousarray(i1b.reshape(H, 1), np.float32),
        "i2s": np.ascontiguousarray(i2s, np.float32),
        "b2i": np.ascontiguousarray(i2b.reshape(H, 1), np.float32),
        "i3s": np.ascontiguousarray(i3s, np.float32),
        "b3ih": np.ascontiguousarray(b3ih.reshape(H, 1), np.float32),
        "rwh": np.ascontiguousarray(rwh, np.float32),
    }


def kernel(ys_coeffs, init_params, func_params, readout_W, readout_b,
           _trace=False, _tmpdir=None):
    ys = np.asarray(ys_coeffs, np.float32)
    assert ys.shape == (B, T, C), ys.shape
    rb = float(np.asarray(readout_b, np.float32).reshape(-1)[0])

    consts = _prep_consts(init_params, func_params, readout_W)
    nc = _get_program()

    in_maps = []
    for cid in range(NCORES):
        m = dict(consts)
        m["ys"] = np.ascontiguousarray(
            ys[cid * BC:(cid + 1) * BC].reshape(BC, T * C))
        in_maps.append(m)

    kw = {}
    if _trace:
        kw = dict(trace=True, tmpdir=_tmpdir)
    res = run_bass_kernel_spmd(nc, in_maps, core_ids=list(range(NCORES)), **kw)
    total = sum(float(r["partial"][0, 0]) for r in res.results)
    out = np.float32(total / B + rb)
    if _trace:
        return np.asarray(out, np.float32), res
    return np.asarray(out, np.float32)
